# revision 1
# baseline (speedup 1.0000x reference)
"""Multi-head attention Bass/Tile kernel for TRN2, 8-core SPMD.

Sharding: core c handles batch b = c//2, query-half qh = c%2. The host
rotates the token axis per core so query rows sit at [0:TQ] (attention is
key-permutation invariant), and gathers the unmasked keys (mask compaction)
so K/V projection + attention only touch TK <= T key tokens.

Each core: Q proj for its TQ query rows, K/V proj for the TK compacted
keys of its batch (duplicated within the batch pair), attention (softmax
without max-subtraction — scores are O(3) here; padded keys get an exp
bias of -1e30), out-proj, residual + LayerNorm.
Output per core: [TQ, D] f32 slice; host assembles [T, B, D].

Matmul layouts (out = lhsT.T @ rhs, contraction on partitions):
  QT/KT [F, *] bf16 : lhsT=w*T [D,F] chunks, rhs=hT* [D,*] chunks
  V     [TK, F] bf16: lhsT=hTk chunk [D, t128], rhs=wvT [D, F]
  S^T   [j, (h0 i512 | h1 i512)] psum (2 banks): row-tiled head pair
  exp   one ACT op per j-tile: [128, 1024], bias=maskbias per-partition
  PV+den [d0:64|den 64:128, i] psum: lhsT=V[j,64]@(0,0) + ones[j,64]@(0,64)
  O     [t, D] psum : lhsT=AVT [f, t128], rhs=woT [f, D]
"""
import numpy as np
import ml_dtypes

import concourse.bass as bass
import concourse.tile as tile
from concourse import bacc, mybir

F32 = mybir.dt.float32
BF16 = mybir.dt.bfloat16
AF = mybir.ActivationFunctionType
ALU = mybir.AluOpType

NEG_BIG = -1.0e30


def _pin_act_tables():
    """Force every ACT func we use (Exp, Ln, Square, Copy) to resolve to
    the single `natural_log_exp_and_others` table set, so the kernel does
    exactly one ACT_TABLE_LOAD instead of thrashing (~2.6us per switch).
    Preserves dict order (set ids are positional)."""
    import concourse.hw_specs as hw_specs
    if getattr(hw_specs, "_mha_tables_pinned", False):
        return
    orig = hw_specs.get_activation_tables

    def patched(module_arch):
        tabs = orig(module_arch)
        pin = "natural_log_exp_and_others"
        if pin in tabs:
            pinned_funcs = tabs[pin]
            for name, fns in tabs.items():
                if name != pin:
                    tabs[name] = fns - pinned_funcs
        return tabs

    hw_specs.get_activation_tables = patched
    import concourse.bacc as bacc_mod
    bacc_mod.get_activation_tables = patched
    hw_specs._mha_tables_pinned = True


def _chunks(total, step):
    out = []
    off = 0
    while off < total:
        out.append((off, min(step, total - off)))
        off += step
    return out


def build_nc(T, TQ, TK, D, NH, DH, n_cores=8, debug=False):
    """Build the single-core SPMD Bass program. TK = compacted key count."""
    F = NH * DH
    DC = D // 128        # D contraction chunks
    FC = F // 128        # feature chunks (2 heads per chunk, DH=64)
    KC = TK // 128       # key tiles
    TT = TQ // 128       # query t-tiles
    ICS = min(512, TQ)   # i-chunk size
    ICN = TQ // ICS
    FS = min(512, F)
    DS = min(512, D)
    assert DH == 64 and F % 128 == 0 and D % 128 == 0
    assert TQ % 128 == 0 and TK % 128 == 0

    _pin_act_tables()
    nc = bacc.Bacc("TRN2", target_bir_lowering=False, debug=debug,
                   num_devices=n_cores)

    # ---- DRAM I/O ----
    hTq_d = nc.dram_tensor("hTq", [DC * 128, TQ], BF16, kind="ExternalInput")
    hTk_d = nc.dram_tensor("hTk", [DC * 128, TK], BF16, kind="ExternalInput")
    hq_d = nc.dram_tensor("hq", [TQ, D], F32, kind="ExternalInput")
    wqT_d = nc.dram_tensor("wqT", [DC * 128, F], BF16, kind="ExternalInput")
    wkT_d = nc.dram_tensor("wkT", [DC * 128, F], BF16, kind="ExternalInput")
    wvT_d = nc.dram_tensor("wvT", [DC * 128, F], BF16, kind="ExternalInput")
    woT_d = nc.dram_tensor("woT", [FC * 128, D], BF16, kind="ExternalInput")
    mb_d = nc.dram_tensor("maskbias", [128, KC], F32, kind="ExternalInput")
    g_d = nc.dram_tensor("g_rep", [128, D], F32, kind="ExternalInput")
    b_d = nc.dram_tensor("b_rep", [128, D], F32, kind="ExternalInput")
    out_d = nc.dram_tensor("out", [TQ, D], F32, kind="ExternalOutput")

    with tile.TileContext(nc) as tc:
        with (
            tc.tile_pool(name="hpool", bufs=1) as hpool,
            tc.tile_pool(name="wts", bufs=2) as wts,
            tc.tile_pool(name="acts", bufs=1) as acts,
            tc.tile_pool(name="small", bufs=1) as small,
            tc.tile_pool(name="exps", bufs=10) as expp,
            tc.tile_pool(name="epi", bufs=3) as epi,
            tc.tile_pool(name="psA", bufs=3, space="PSUM") as psA,
            tc.tile_pool(name="psB", bufs=2, space="PSUM") as psB,
        ):
            # ---- persistent SBUF tiles ----
            hTq = hpool.tile([128, DC * TQ], BF16, tag="htq")
            hTk = hpool.tile([128, DC * TK], BF16, tag="htk")
            wqT = wts.tile([128, DC * F], BF16, tag="w")
            wkT = wts.tile([128, DC * F], BF16, tag="w")
            wvT = wts.tile([128, DC * F], BF16, tag="w")
            QT = acts.tile([128, FC * TQ], BF16, tag="qt")
            KT = acts.tile([128, FC * TK], BF16, tag="kt")
            V = acts.tile([128, KC * F], BF16, tag="v")
            AVT = acts.tile([128, FC * TQ], BF16, tag="avt")
            ones = small.tile([128, 64], BF16, tag="ones")
            mb = small.tile([128, KC], F32, tag="mb")
            eps_t = small.tile([128, 1], F32, tag="eps")

            nc.vector.memset(ones[:], 1.0)
            nc.vector.memset(eps_t[:], 1e-5)
            nc.sync.dma_start(mb[:], mb_d[:])
            # DMA in consumption order: the first QT matmul chain needs
            # wqT[dc=0] + hTq[dc=0] first; V's wvT comes last.
            for dc in range(DC):
                nc.sync.dma_start(wqT[:, dc * F:(dc + 1) * F],
                                  wqT_d[dc * 128:(dc + 1) * 128, :])
                nc.sync.dma_start(hTq[:, dc * TQ:(dc + 1) * TQ],
                                  hTq_d[dc * 128:(dc + 1) * 128, :])
            for dc in range(DC):
                nc.sync.dma_start(wkT[:, dc * F:(dc + 1) * F],
                                  wkT_d[dc * 128:(dc + 1) * 128, :])
                nc.sync.dma_start(hTk[:, dc * TK:(dc + 1) * TK],
                                  hTk_d[dc * 128:(dc + 1) * 128, :])
            for dc in range(DC):
                nc.sync.dma_start(wvT[:, dc * F:(dc + 1) * F],
                                  wvT_d[dc * 128:(dc + 1) * 128, :])

            # ---- stage 1: projections ----
            for p in range(FC):     # QT chunks [128f, TQ]
                for t0, tn in _chunks(TQ, 512):
                    ps = psA.tile([128, tn], F32, tag="A")
                    for dc in range(DC):
                        nc.tensor.matmul(
                            ps[:],
                            wqT[:, dc * F + p * 128: dc * F + (p + 1) * 128],
                            hTq[:, dc * TQ + t0: dc * TQ + t0 + tn],
                            start=(dc == 0), stop=(dc == DC - 1))
                    nc.vector.tensor_copy(
                        QT[:, p * TQ + t0: p * TQ + t0 + tn], ps[:])
            for p in range(FC):     # KT chunks [128f, TK]
                for t0, tn in _chunks(TK, 512):
                    ps = psA.tile([128, tn], F32, tag="A")
                    for dc in range(DC):
                        nc.tensor.matmul(
                            ps[:],
                            wkT[:, dc * F + p * 128: dc * F + (p + 1) * 128],
                            hTk[:, dc * TK + t0: dc * TK + t0 + tn],
                            start=(dc == 0), stop=(dc == DC - 1))
                    nc.scalar.copy(
                        KT[:, p * TK + t0: p * TK + t0 + tn], ps[:])
            def emit_v_chain(jc, f0, fn):
                ps = psA.tile([128, fn], F32, tag="A")
                for dc in range(DC):
                    nc.tensor.matmul(
                        ps[:],
                        hTk[:, dc * TK + jc * 128: dc * TK + (jc + 1) * 128],
                        wvT[:, dc * F + f0: dc * F + f0 + fn],
                        start=(dc == 0), stop=(dc == DC - 1))
                nc.vector.tensor_copy(
                    V[:, jc * F + f0: jc * F + f0 + fn], ps[:])

            for jc in range(KC):
                for f0, fn in _chunks(F, FS):
                    emit_v_chain(jc, f0, fn)

            # late loads (overlap with attention)
            woT = wts.tile([128, FC * D], BF16, tag="w")
            for fc_ in range(FC):
                nc.sync.dma_start(woT[:, fc_ * D:(fc_ + 1) * D],
                                  woT_d[fc_ * 128:(fc_ + 1) * 128, :])
            g_re = small.tile([128, D], F32, tag="g")
            b_re = small.tile([128, D], F32, tag="b")
            nc.sync.dma_start(g_re[:], g_d[:])
            nc.sync.dma_start(b_re[:], b_d[:])

            # ---- stage 2+3 interleaved: attention, then out-proj+LN
            # for each query i-chunk so the LN tail overlaps attention ----
            def oln_tile(tt):
                x = epi.tile([128, D], F32, tag="x")
                hqt = epi.tile([128, D], F32, tag="hqt")
                nc.sync.dma_start(hqt[:], hq_d[tt * 128:(tt + 1) * 128, :])
                for d0, dn in _chunks(D, DS):
                    ps = psA.tile([128, dn], F32, tag="A")
                    for fc_ in range(FC):
                        nc.tensor.matmul(
                            ps[:],
                            AVT[:, fc_ * TQ + tt * 128: fc_ * TQ + (tt + 1) * 128],
                            woT[:, fc_ * D + d0: fc_ * D + d0 + dn],
                            start=(fc_ == 0), stop=(fc_ == FC - 1))
                    nc.vector.tensor_tensor(
                        x[:, d0:d0 + dn], ps[:],
                        hqt[:, d0:d0 + dn], op=ALU.add)
                stats = epi.tile([128, 4], F32, tag="stats")
                xc = epi.tile([128, D], F32, tag="xc")
                # mean-sum on ACT (Identity+accum; xc is scratch here)
                nc.scalar.activation(xc[:], x[:], AF.Identity,
                                     accum_out=stats[:, 0:1])
                negmu = stats[:, 1:2]
                nc.vector.tensor_scalar(negmu, stats[:, 0:1], -1.0 / D, None,
                                        op0=ALU.mult)
                # xc = x - mu on ACT (Identity with per-partition bias)
                nc.scalar.activation(xc[:], x[:], AF.Identity, bias=negmu)
                # var-sum on ACT (Square+accum; x is dead scratch)
                nc.scalar.activation(x[:], xc[:], AF.Square,
                                     accum_out=stats[:, 2:3])
                var = stats[:, 3:4]
                nc.vector.tensor_scalar(var, stats[:, 2:3], 1.0 / D, None,
                                        op0=ALU.mult)
                # rstd = exp(-0.5*ln(var+eps)): Ln+Exp live in one ACT
                # table set with the attention Exps -> no table thrash
                lnv = stats[:, 0:1]
                nc.scalar.activation(lnv, var, AF.Ln, bias=eps_t[:])
                rstd = stats[:, 1:2]
                nc.scalar.activation(rstd, lnv, AF.Exp, scale=-0.5)
                nc.vector.scalar_tensor_tensor(xc[:], xc[:], rstd, g_re[:],
                                               op0=ALU.mult, op1=ALU.mult)
                nc.vector.tensor_tensor(xc[:], xc[:], b_re[:], op=ALU.add)
                nc.sync.dma_start(out_d[tt * 128:(tt + 1) * 128, :], xc[:])

            for ic in range(ICN):
                io = ic * ICS
                for hp in range(FC):
                    h0, h1 = 2 * hp, 2 * hp + 1
                    pvP = psB.tile([128, ICS], F32, tag="pv")
                    pvD = psB.tile([128, ICS], F32, tag="pv")
                    for jc in range(KC):
                        # S^T pair: [j, h0-i | h1-i] across 2 psum banks
                        s = psA.tile([128, 2 * ICS], F32, tag="A")
                        nc.tensor.matmul(
                            s[:, 0:ICS],
                            KT[0:64, hp * TK + jc * 128: hp * TK + (jc + 1) * 128],
                            QT[0:64, hp * TQ + io: hp * TQ + io + ICS],
                            start=True, stop=True, tile_position=(0, 0))
                        nc.tensor.matmul(
                            s[:, ICS:2 * ICS],
                            KT[64:128, hp * TK + jc * 128: hp * TK + (jc + 1) * 128],
                            QT[64:128, hp * TQ + io: hp * TQ + io + ICS],
                            start=True, stop=True, tile_position=(64, 0))
                        e = expp.tile([128, 2 * ICS], BF16, tag="e")
                        nc.scalar.activation(e[:], s[:], AF.Exp,
                                             bias=mb[:, jc:jc + 1])
                        st, sp = (jc == 0), (jc == KC - 1)
                        nc.tensor.matmul(
                            pvP[0:64, :],
                            V[:, jc * F + h0 * DH: jc * F + (h0 + 1) * DH],
                            e[:, 0:ICS], start=st, stop=sp,
                            tile_position=(0, 0), skip_group_check=True)
                        nc.tensor.matmul(
                            pvP[64:128, :],
                            V[:, jc * F + h1 * DH: jc * F + (h1 + 1) * DH],
                            e[:, ICS:2 * ICS], start=st, stop=sp,
                            tile_position=(0, 64), skip_group_check=True)
                        nc.tensor.matmul(
                            pvD[0:64, :], ones[:, 0:64],
                            e[:, 0:ICS], start=st, stop=sp,
                            tile_position=(0, 0), skip_group_check=True)
                        nc.tensor.matmul(
                            pvD[64:128, :], ones[:, 0:64],
                            e[:, ICS:2 * ICS], start=st, stop=sp,
                            tile_position=(0, 64), skip_group_check=True)
                    # normalize -> AVT: copy PV/den to SBUF fast (frees
                    # the PSUM banks for the next head pair), then one
                    # reciprocal + one multiply covering both heads.
                    pvPc = epi.tile([128, ICS], F32, tag="rec")
                    pvDc = epi.tile([128, ICS], F32, tag="rec")
                    nc.vector.tensor_copy(pvPc[:], pvP[:])
                    nc.vector.tensor_copy(pvDc[:], pvD[:])
                    nc.vector.reciprocal(pvDc[:], pvDc[:])
                    nc.vector.tensor_tensor(
                        AVT[:, hp * TQ + io: hp * TQ + io + ICS],
                        pvPc[:], pvDc[:], op=ALU.mult)

                # out-proj + residual + LayerNorm for this ic's t-tiles
                for tt in range(io // 128, (io + ICS) // 128):
                    oln_tile(tt)

    nc.compile()
    return nc


def choose_tk(attn_mask):
    """Compacted key count: max unmasked count over batches, ceil to 128."""
    m = np.asarray(attn_mask)
    counts = (~m).sum(axis=0)
    tk = int(((int(counts.max()) + 127) // 128) * 128)
    return max(tk, 128)


def host_prep_core(c, tk, h, attn_mask, wq, wkv, wo, ln_g, ln_b, NH=16, DH=64):
    """Build the per-core input map (numpy) for core c."""
    T, B, D = h.shape
    F = NH * DH
    TQ = T // 2
    KC = tk // 128
    b, qh = c // 2, c % 2
    bf = ml_dtypes.bfloat16
    hb = np.roll(np.asarray(h[:, b, :], dtype=np.float32), -qh * TQ, axis=0)
    maskb = np.roll(np.asarray(attn_mask[:, b]), -qh * TQ)
    idx = np.nonzero(~maskb)[0]
    nk = idx.shape[0]
    assert nk <= tk
    idxp = np.concatenate([idx, np.zeros(tk - nk, np.int64)])
    scale = 1.0 / np.sqrt(DH)
    hbT = np.ascontiguousarray(hb.T).astype(bf)             # [D, T]
    m = {}
    m["hTq"] = np.ascontiguousarray(hbT[:, :TQ])
    m["hTk"] = np.ascontiguousarray(hbT[:, idxp])
    m["hq"] = np.ascontiguousarray(hb[:TQ])                 # [TQ, D] f32
    m["wqT"] = np.ascontiguousarray(wq.T * scale).astype(bf)
    m["wkT"] = np.ascontiguousarray(wkv[:F].T).astype(bf)
    m["wvT"] = np.ascontiguousarray(wkv[F:].T).astype(bf)
    m["woT"] = np.ascontiguousarray(wo.T).astype(bf)
    mbias = np.full(tk, NEG_BIG, np.float32)
    mbias[:nk] = 0.0
    m["maskbias"] = np.ascontiguousarray(mbias.reshape(KC, 128).T)
    m["g_rep"] = np.ascontiguousarray(
        np.broadcast_to(np.asarray(ln_g, np.float32), (128, D)))
    m["b_rep"] = np.ascontiguousarray(
        np.broadcast_to(np.asarray(ln_b, np.float32), (128, D)))
    return m

# ======================================================================
# Host-side runner: shard, compile (cached), execute on 8 cores, gather.
# ======================================================================
_NC_CACHE = {}
LAST_RESULT = None  # BassKernelResults of the most recent kernel() call


def _get_nc(T, TQ, TK, D, NH, DH):
    key = (T, TQ, TK, D, NH, DH)
    if key not in _NC_CACHE:
        _NC_CACHE[key] = build_nc(T, TQ, TK, D, NH, DH, n_cores=8, debug=False)
    return _NC_CACHE[key]


def kernel(h, attn_mask, wq, wkv, wo, ln_g, ln_b):
    """Full-input MultiHeadAttn forward on 8 NeuronCores.

    h: [T, B, D] f32; attn_mask: [T, B] bool (True = masked key);
    wq: [F, D]; wkv: [2F, D]; wo: [D, F]; ln_g/ln_b: [D].
    Returns [T, B, D] f32 = layer_norm(h + attn(h)).
    """
    from concourse.bass_utils import run_bass_kernel_spmd
    global LAST_RESULT

    h = np.asarray(h)
    attn_mask = np.asarray(attn_mask)
    wq = np.asarray(wq, np.float32)
    wkv = np.asarray(wkv, np.float32)
    wo = np.asarray(wo, np.float32)
    ln_g = np.asarray(ln_g, np.float32)
    ln_b = np.asarray(ln_b, np.float32)

    T, B, D = h.shape
    NH = 16
    DH = wq.shape[0] // NH
    assert 2 * B == 8, "sharding assumes batch 4 over 8 cores"
    TQ = T // 2
    TK = min(choose_tk(attn_mask), T)

    nc = _get_nc(T, TQ, TK, D, NH, DH)
    in_maps = [host_prep_core(c, TK, h, attn_mask, wq, wkv, wo, ln_g, ln_b,
                              NH=NH, DH=DH) for c in range(8)]
    res = run_bass_kernel_spmd(nc, in_maps, core_ids=list(range(8)))
    LAST_RESULT = res

    out = np.empty((T, B, D), np.float32)
    for c in range(8):
        b, qh = c // 2, c % 2
        out[qh * TQ:(qh + 1) * TQ, b, :] = res.results[c]["out"]
    return out



# revision 13
# speedup vs baseline: 1.0382x; 1.0382x over previous
"""Multi-head attention Bass/Tile kernel for TRN2, 8-core SPMD.

Sharding: core c handles batch b = c//2, query-half qh = c%2. The host
rotates the token axis per core so query rows sit at [0:TQ] (attention is
key-permutation invariant), and gathers the unmasked keys (mask compaction)
so K/V projection + attention only touch TK <= T key tokens.

v2: software-pipelined emission. The projection matmul chains (Q/K/V/O)
are drained a few MMs at a time between the attention matmuls so ScalarE
(softmax exp, the #2 engine) starts ~15us into the kernel instead of
~95us, and TensorE never idles waiting on exp. Other changes vs v1:
  - softmax denominator reciprocal via reciprocal_approx_fast (~5x faster
    than the iterative DVE reciprocal that cost 3.3us per (ic,hp) unit)
  - LayerNorm mean/var via DVE bn_stats/bn_aggr instead of ScalarE
    Identity/Square+accum passes; ScalarE does only exp + 2 tiny ops/tile
  - AVT normalize multiply reads PV straight from PSUM (no copy)
  - unit order (ic,hp): ic0/h0-3, ic1/h0-3, ic0/h4-7, ic1/h4-7 spreads
    the projection work evenly across the attention timeline

Matmul layouts (out = lhsT.T @ rhs, contraction on partitions):
  QT/KT [F, *] bf16 : lhsT=w*T [D,F] chunks, rhs=hT* [D,*] chunks
  V     [TK, F] bf16: lhsT=hTk chunk [D, t128], rhs=wvT [D, F]
  S^T   [j, (h0 i512 | h1 i512)] psum (2 banks): row-tiled head pair
  exp   one ACT op per j-tile: [128, 1024], bias=maskbias per-partition
  PV+den [d0:64|den 64:128, i] psum: col-tiled pairs, ones-matmul denom
  O     [t, D] psum : lhsT=AVT [f, t128], rhs=woT [f, D]
"""
from collections import deque
from types import SimpleNamespace

import numpy as np
import ml_dtypes

import concourse.bass as bass
import concourse.tile as tile
from concourse import bacc, mybir

F32 = mybir.dt.float32
BF16 = mybir.dt.bfloat16
AF = mybir.ActivationFunctionType
ALU = mybir.AluOpType

NEG_BIG = -1.0e30


def _pin_act_tables():
    """Force every ACT func we use (Exp, Ln) to resolve to the single
    `natural_log_exp_and_others` table set, so the kernel does exactly one
    ACT_TABLE_LOAD instead of thrashing (~2.6us per switch)."""
    import concourse.hw_specs as hw_specs
    if getattr(hw_specs, "_mha_tables_pinned", False):
        return
    orig = hw_specs.get_activation_tables

    def patched(module_arch):
        tabs = orig(module_arch)
        pin = "natural_log_exp_and_others"
        if pin in tabs:
            pinned_funcs = tabs[pin]
            for name, fns in tabs.items():
                if name != pin:
                    tabs[name] = fns - pinned_funcs
        return tabs

    hw_specs.get_activation_tables = patched
    import concourse.bacc as bacc_mod
    bacc_mod.get_activation_tables = patched
    hw_specs._mha_tables_pinned = True


def _chunks(total, step):
    out = []
    off = 0
    while off < total:
        out.append((off, min(step, total - off)))
        off += step
    return out


def build_nc(T, TQ, TK, D, NH, DH, n_cores=8, debug=False):
    """Build the single-core SPMD Bass program. TK = compacted key count."""
    F = NH * DH
    DC = D // 128        # D contraction chunks
    FC = F // 128        # feature chunks (2 heads per chunk, DH=64)
    KC = TK // 128       # key tiles
    ICS = min(512, TQ)   # i-chunk size
    ICN = TQ // ICS
    FH = F // 2          # feature half (V is projected in 2 halves)
    assert DH == 64 and F % 128 == 0 and D % 128 == 0
    assert TQ % 128 == 0 and TK % 128 == 0 and ICN == 2 and FC == 8

    _pin_act_tables()
    nc = bacc.Bacc("TRN2", target_bir_lowering=False, debug=debug,
                   num_devices=n_cores)

    # ---- DRAM I/O ----
    hTq_d = nc.dram_tensor("hTq", [DC * 128, TQ], BF16, kind="ExternalInput")
    hTk_d = nc.dram_tensor("hTk", [DC * 128, TK], BF16, kind="ExternalInput")
    hq_d = nc.dram_tensor("hq", [TQ, D], F32, kind="ExternalInput")
    wqT_d = nc.dram_tensor("wqT", [DC * 128, F], BF16, kind="ExternalInput")
    wkT_d = nc.dram_tensor("wkT", [DC * 128, F], BF16, kind="ExternalInput")
    wvT_d = nc.dram_tensor("wvT", [DC * 128, F], BF16, kind="ExternalInput")
    woT_d = nc.dram_tensor("woT", [FC * 128, D], BF16, kind="ExternalInput")
    mb_d = nc.dram_tensor("maskbias", [128, KC], F32, kind="ExternalInput")
    g_d = nc.dram_tensor("g_rep", [128, D], F32, kind="ExternalInput")
    b_d = nc.dram_tensor("b_rep", [128, D], F32, kind="ExternalInput")
    out_d = nc.dram_tensor("out", [TQ, D], F32, kind="ExternalOutput")

    with tile.TileContext(nc) as tc:
        with (
            tc.tile_pool(name="big", bufs=1) as big,
            tc.tile_pool(name="ep", bufs=4) as ep,
            tc.tile_pool(name="hqp", bufs=2) as hqp,
            tc.tile_pool(name="lnp", bufs=2) as lnp,
            tc.tile_pool(name="pvs", bufs=1) as pvs,
            tc.tile_pool(name="statp", bufs=2) as statp,
            tc.tile_pool(name="psS", bufs=2, space="PSUM") as psS,
            tc.tile_pool(name="psPV", bufs=2, space="PSUM") as psPV,
            tc.tile_pool(name="psP", bufs=2, space="PSUM") as psP,
        ):
            # ---- persistent SBUF tiles ----
            hTq = big.tile([128, DC * TQ], BF16, tag="htq")
            hTk = big.tile([128, DC * TK], BF16, tag="htk")
            wqT = big.tile([128, DC * F], BF16, tag="wq")
            wkT = big.tile([128, DC * F], BF16, tag="wk")
            wvT = big.tile([128, DC * F], BF16, tag="wv")
            woT = big.tile([128, FC * D], BF16, tag="wo")
            QT = big.tile([128, FC * TQ], BF16, tag="qt")
            KT = big.tile([128, FC * TK], BF16, tag="kt")
            V = big.tile([128, KC * F], BF16, tag="v")
            AVT = big.tile([128, FC * TQ], BF16, tag="avt")
            g_re = big.tile([128, D], F32, tag="g")
            b_re = big.tile([128, D], F32, tag="b")
            ones = big.tile([128, 64], BF16, tag="ones")
            mb = big.tile([128, KC], F32, tag="mb")
            eps_t = big.tile([128, 1], F32, tag="eps")

            nc.vector.memset(ones[:], 1.0)
            nc.vector.memset(eps_t[:], 1e-5)
            nc.sync.dma_start(mb[:], mb_d[:])

            # ---- DMA in consumption order ----
            # First wave: what the prologue + first unit touch.
            for dc in range(DC):
                nc.sync.dma_start(wqT[:, dc * F:(dc + 1) * F],
                                  wqT_d[dc * 128:(dc + 1) * 128, :])
            for dc in range(DC):  # hTq i0 half (Q(0,i0) rhs)
                nc.sync.dma_start(hTq[:, dc * TQ: dc * TQ + ICS],
                                  hTq_d[dc * 128:(dc + 1) * 128, 0:ICS])
            for dc in range(DC):  # wk cols of hp0
                nc.sync.dma_start(wkT[:, dc * F: dc * F + 128],
                                  wkT_d[dc * 128:(dc + 1) * 128, 0:128])
            hTk_half = min(512, TK)
            for dc in range(DC):  # hTk first 512 keys
                nc.sync.dma_start(hTk[:, dc * TK: dc * TK + hTk_half],
                                  hTk_d[dc * 128:(dc + 1) * 128, 0:hTk_half])
            for dc in range(DC):  # wv first feature half
                nc.sync.dma_start(wvT[:, dc * F: dc * F + FH],
                                  wvT_d[dc * 128:(dc + 1) * 128, 0:FH])
            # Second wave: the rest of the preload.
            for dc in range(DC):
                nc.sync.dma_start(hTq[:, dc * TQ + ICS: dc * TQ + TQ],
                                  hTq_d[dc * 128:(dc + 1) * 128, ICS:TQ])
                nc.sync.dma_start(hTk[:, dc * TK + hTk_half: dc * TK + TK],
                                  hTk_d[dc * 128:(dc + 1) * 128, hTk_half:TK])
                nc.sync.dma_start(wkT[:, dc * F + 128:(dc + 1) * F],
                                  wkT_d[dc * 128:(dc + 1) * 128, 128:F])
                nc.sync.dma_start(wvT[:, dc * F + FH:(dc + 1) * F],
                                  wvT_d[dc * 128:(dc + 1) * 128, FH:F])

            # ================= projection chain machinery =================
            def q_chain(hp, icx):
                io = icx * ICS
                ch = SimpleNamespace(ps=None)

                def mm(dc):
                    def f():
                        if ch.ps is None:
                            ch.ps = psP.tile([128, ICS], F32, tag="P",
                                             name="qps")
                        nc.tensor.matmul(
                            ch.ps[:],
                            wqT[:, dc * F + hp * 128: dc * F + (hp + 1) * 128],
                            hTq[:, dc * TQ + io: dc * TQ + io + ICS],
                            start=(dc == 0), stop=(dc == DC - 1),
                            skip_group_check=True)
                    return f

                def fin():
                    nc.vector.tensor_copy(
                        QT[:, hp * TQ + io: hp * TQ + io + ICS], ch.ps[:])
                return SimpleNamespace(mms=deque(mm(dc) for dc in range(DC)),
                                       fin=fin)

            def k_chain(hp, c0, cn):
                ch = SimpleNamespace(ps=None)

                def mm(dc):
                    def f():
                        if ch.ps is None:
                            ch.ps = psP.tile([128, cn], F32, tag="P",
                                             name="kps")
                        nc.tensor.matmul(
                            ch.ps[:],
                            wkT[:, dc * F + hp * 128: dc * F + (hp + 1) * 128],
                            hTk[:, dc * TK + c0: dc * TK + c0 + cn],
                            start=(dc == 0), stop=(dc == DC - 1),
                            skip_group_check=True)
                    return f

                def fin():
                    nc.vector.tensor_copy(
                        KT[:, hp * TK + c0: hp * TK + c0 + cn], ch.ps[:])
                return SimpleNamespace(mms=deque(mm(dc) for dc in range(DC)),
                                       fin=fin)

            def v_chain(jc, fh):
                f0 = fh * FH
                ch = SimpleNamespace(ps=None)

                def mm(dc):
                    def f():
                        if ch.ps is None:
                            ch.ps = psP.tile([128, FH], F32, tag="P",
                                             name="vps")
                        nc.tensor.matmul(
                            ch.ps[:],
                            hTk[:, dc * TK + jc * 128: dc * TK + (jc + 1) * 128],
                            wvT[:, dc * F + f0: dc * F + f0 + FH],
                            start=(dc == 0), stop=(dc == DC - 1),
                            skip_group_check=True)
                    return f

                def fin():
                    nc.vector.tensor_copy(
                        V[:, jc * F + f0: jc * F + f0 + FH], ch.ps[:])
                return SimpleNamespace(mms=deque(mm(dc) for dc in range(DC)),
                                       fin=fin)

            # O-proj + residual + LayerNorm, per t-tile of 128 queries.
            tt_state = {}

            def ln_tail(tt, st):
                st6 = statp.tile([128, 12], F32, tag="st6")
                nc.vector.bn_stats(st6[:, 0:6], st.x[:, 0:512])
                nc.vector.bn_stats(st6[:, 6:12], st.x[:, 512:1024])
                mv = statp.tile([128, 2], F32, tag="mv")
                nc.vector.bn_aggr(mv[:], st6[:])
                lnv = statp.tile([128, 1], F32, tag="lnv")
                rstd = statp.tile([128, 1], F32, tag="rstd")
                # rstd = exp(-0.5*ln(var+eps)); Ln+Exp share one ACT table set
                nc.scalar.activation(lnv[:], mv[:, 1:2], AF.Ln, bias=eps_t[:])
                nc.scalar.activation(rstd[:], lnv[:], AF.Exp, scale=-0.5)
                xc = lnp.tile([128, D], F32, tag="xc", bufs=1)
                nc.vector.scalar_tensor_tensor(
                    xc[:], st.x[:], mv[:, 0:1], g_re[:],
                    op0=ALU.subtract, op1=ALU.mult)
                # write into the dead hqt tile, then DMA out
                nc.vector.scalar_tensor_tensor(
                    st.hqt[:], xc[:], rstd[:], b_re[:],
                    op0=ALU.mult, op1=ALU.add)
                nc.sync.dma_start(out_d[tt * 128:(tt + 1) * 128, :], st.hqt[:])

            def o_chain(tt, dci):
                d0 = dci * 512
                ch = SimpleNamespace(ps=None)

                def mm(fc):
                    def f():
                        if ch.ps is None:
                            if tt not in tt_state:
                                hqt = hqp.tile([128, D], F32, tag="hq")
                                nc.sync.dma_start(
                                    hqt[:], hq_d[tt * 128:(tt + 1) * 128, :])
                                x = lnp.tile([128, D], F32, tag="x")
                                tt_state[tt] = SimpleNamespace(
                                    hqt=hqt, x=x, done=0)
                            ch.ps = psP.tile([128, 512], F32, tag="P",
                                             name="ops")
                        nc.tensor.matmul(
                            ch.ps[:],
                            AVT[:, fc * TQ + tt * 128: fc * TQ + (tt + 1) * 128],
                            woT[:, fc * D + d0: fc * D + d0 + 512],
                            start=(fc == 0), stop=(fc == FC - 1),
                            skip_group_check=True)
                    return f

                def fin():
                    st = tt_state[tt]
                    nc.vector.tensor_tensor(
                        st.x[:, d0:d0 + 512], ch.ps[:],
                        st.hqt[:, d0:d0 + 512], op=ALU.add)
                    st.done += 1
                    if st.done == 2:
                        ln_tail(tt, st)
                return SimpleNamespace(mms=deque(mm(fc) for fc in range(FC)),
                                       fin=fin)

            # ---- build the drain queue in deadline order ----
            # Every chain carries a `deadline` (unit_idx, loop_iter): its data
            # is consumed by the attention emission at that position, so it
            # must be FULLY emitted before then (Tile tracks deps in program
            # order — a read emitted before its producing write races).
            # Chains whose matmuls depend on the attention pipeline itself
            # (the O-proj chains reading AVT) carry a `gate`: pump() must not
            # emit them until that many units are fully emitted, else the
            # in-order PE queue deadlocks on its own later instructions.
            # unit order (ic, hp); index in this list is the deadline key.
            units = [(0, 0), (0, 1), (0, 2), (0, 3),
                     (1, 0), (1, 1), (1, 2), (1, 3),
                     (0, 4), (0, 5), (0, 6), (0, 7),
                     (1, 4), (1, 5), (1, 6), (1, 7)]
            uidx_of = {u: i for i, u in enumerate(units)}
            LAG = 2
            FAR = (99, 0)

            kchunks = _chunks(TK, 512)
            queue = deque()
            units_done = SimpleNamespace(n=0)
            qmm = SimpleNamespace(n=0)

            def push(ch, deadline=FAR, gate=0):
                ch.gate = gate
                ch.deadline = deadline
                qmm.n += len(ch.mms)
                queue.append(ch)

            def k_dl(hp, ci):
                return (uidx_of[(0, hp)], (kchunks[ci][0]) // 128)

            def q_dl(hp, icx):
                return (uidx_of[(icx, hp)], 0)

            def v_dl(jc, fh):
                return (uidx_of[(0, 0 if fh == 0 else 4)], jc + LAG)

            # u1 needs: V(4..KC-1, fh0), KT(0,c1)
            fh0_rest = list(range(4, KC))
            if fh0_rest:
                push(v_chain(fh0_rest[0], 0), v_dl(fh0_rest[0], 0))
            for ci in range(1, len(kchunks)):
                push(k_chain(0, *kchunks[ci]), k_dl(0, ci))
            for jc in fh0_rest[1:]:
                push(v_chain(jc, 0), v_dl(jc, 0))
            # u2..u4: KT/QT for hp 1..3 (+ first V fh1 chains)
            for hp in (1, 2, 3):
                push(k_chain(hp, *kchunks[0]), k_dl(hp, 0))
                push(q_chain(hp, 0), q_dl(hp, 0))
                for ci in range(1, len(kchunks)):
                    push(k_chain(hp, *kchunks[ci]), k_dl(hp, ci))
                if hp >= 2:
                    push(v_chain(hp - 2, 1), v_dl(hp - 2, 1))
            # u5..u8: QT(0..3, i1), rest of V fh1, KT/QT hp4
            push(q_chain(0, 1), q_dl(0, 1))
            push(v_chain(2, 1), v_dl(2, 1))
            push(q_chain(1, 1), q_dl(1, 1))
            push(v_chain(3, 1), v_dl(3, 1))
            push(q_chain(2, 1), q_dl(2, 1))
            push(v_chain(4 % KC, 1), v_dl(4 % KC, 1))
            push(k_chain(4, *kchunks[0]), k_dl(4, 0))
            push(q_chain(4, 0), q_dl(4, 0))
            push(q_chain(3, 1), q_dl(3, 1))
            for ci in range(1, len(kchunks)):
                push(k_chain(4, *kchunks[ci]), k_dl(4, ci))
            for jc in range(5, KC):
                push(v_chain(jc, 1), v_dl(jc, 1))
            # u9..u12: KT/QT hp 5..7
            for hp in (5, 6, 7):
                push(k_chain(hp, *kchunks[0]), k_dl(hp, 0))
                push(q_chain(hp, 0), q_dl(hp, 0))
                for ci in range(1, len(kchunks)):
                    push(k_chain(hp, *kchunks[ci]), k_dl(hp, ci))
            push(q_chain(4, 1), q_dl(4, 1))
            # u13..u16: QT(5..7, i1) + O(ic0) (gated on AVT(ic0) complete)
            for hp in (5, 6, 7):
                push(q_chain(hp, 1), q_dl(hp, 1))
            for tt in range(0, ICS // 128):
                push(o_chain(tt, 0), FAR, gate=12)
                push(o_chain(tt, 1), FAR, gate=12)
            # tail: O(ic1) (gated on AVT(ic1) complete)
            for tt in range(ICS // 128, TQ // 128):
                push(o_chain(tt, 0), FAR, gate=16)
                push(o_chain(tt, 1), FAR, gate=16)

            def run_rest(ch):
                while ch.mms:
                    ch.mms.popleft()()
                    qmm.n -= 1
                ch.fin()

            def drain_due(pos):
                """Fully emit every queued chain whose deadline has arrived."""
                due = [ch for ch in queue if ch.deadline <= pos]
                for ch in due:
                    queue.remove(ch)
                    run_rest(ch)

            def pump(n):
                while n > 0 and queue:
                    ch = queue[0]
                    if ch.gate > units_done.n:
                        return
                    ch.mms.popleft()()
                    qmm.n -= 1
                    n -= 1
                    if not ch.mms:
                        ch.fin()
                        queue.popleft()

            # ---- prologue projections (emitted whole, never queued) ----
            def run_chain(ch):
                while ch.mms:
                    ch.mms.popleft()()
                ch.fin()

            run_chain(q_chain(0, 0))
            run_chain(k_chain(0, *kchunks[0]))
            for jc in range(min(4, KC)):
                run_chain(v_chain(jc, 0))

            # late loads (overlap with attention)
            for fc_ in range(FC):
                nc.sync.dma_start(woT[:, fc_ * D:(fc_ + 1) * D],
                                  woT_d[fc_ * 128:(fc_ + 1) * 128, :])
            nc.sync.dma_start(g_re[:], g_d[:])
            nc.sync.dma_start(b_re[:], b_d[:])

            # ================= attention units =================
            slots = SimpleNamespace(n=len(units) * KC)

            for uidx, (icx, hp) in enumerate(units):
                io = icx * ICS
                h0, h1 = 2 * hp, 2 * hp + 1
                pvP = psPV.tile([128, ICS], F32, tag="pv", name="pvP")
                pvD = psPV.tile([128, ICS], F32, tag="pv", name="pvD")
                e_tiles = {}

                def emit_S(jc):
                    s = psS.tile([128, 2 * ICS], F32, tag="S", name="s")
                    nc.tensor.matmul(
                        s[:, 0:ICS],
                        KT[0:64, hp * TK + jc * 128: hp * TK + (jc + 1) * 128],
                        QT[0:64, hp * TQ + io: hp * TQ + io + ICS],
                        start=True, stop=True, tile_position=(0, 0),
                        skip_group_check=True)
                    nc.tensor.matmul(
                        s[:, ICS:2 * ICS],
                        KT[64:128, hp * TK + jc * 128: hp * TK + (jc + 1) * 128],
                        QT[64:128, hp * TQ + io: hp * TQ + io + ICS],
                        start=True, stop=True, tile_position=(64, 0),
                        skip_group_check=True)
                    e = ep.tile([128, 2 * ICS], BF16, tag="e", name="e")
                    nc.scalar.activation(e[:], s[:], AF.Exp,
                                         bias=mb[:, jc:jc + 1])
                    e_tiles[jc] = e

                def emit_PV(jc):
                    e = e_tiles.pop(jc)
                    st, sp = (jc == 0), (jc == KC - 1)
                    nc.tensor.matmul(
                        pvP[0:64, :],
                        V[:, jc * F + h0 * DH: jc * F + (h0 + 1) * DH],
                        e[:, 0:ICS], start=st, stop=sp,
                        tile_position=(0, 0), skip_group_check=True)
                    nc.tensor.matmul(
                        pvP[64:128, :],
                        V[:, jc * F + h1 * DH: jc * F + (h1 + 1) * DH],
                        e[:, ICS:2 * ICS], start=st, stop=sp,
                        tile_position=(0, 64), skip_group_check=True)
                    nc.tensor.matmul(
                        pvD[0:64, :], ones[:, 0:64],
                        e[:, 0:ICS], start=st, stop=sp,
                        tile_position=(0, 0), skip_group_check=True)
                    nc.tensor.matmul(
                        pvD[64:128, :], ones[:, 0:64],
                        e[:, ICS:2 * ICS], start=st, stop=sp,
                        tile_position=(0, 64), skip_group_check=True)

                for jc in range(KC + LAG):
                    # force-emit any chain whose consumer is imminent
                    drain_due((uidx, jc))
                    if jc < KC:
                        emit_S(jc)
                        # pace the projection queue across the whole kernel
                        rate = -(-qmm.n // max(slots.n, 1))
                        pump(min(rate, 6))
                        slots.n -= 1
                    else:
                        pump(2)
                    if jc >= LAG:
                        emit_PV(jc - LAG)

                # normalize -> AVT: approx-reciprocal of the denominator,
                # multiply PV (still in PSUM) by it.
                pvDc = pvs.tile([128, ICS], F32, tag="pd")
                nc.vector.tensor_copy(pvDc[:], pvD[:])
                rec = pvs.tile([128, ICS], F32, tag="rc")
                nc.vector.reciprocal_approx_fast(rec[:], pvDc[:])
                nc.vector.tensor_tensor(
                    AVT[:, hp * TQ + io: hp * TQ + io + ICS],
                    pvP[:], rec[:], op=ALU.mult)
                units_done.n += 1

            # drain whatever projection work remains (O(ic1) mostly)
            pump(10 ** 9)

    nc.compile()
    return nc


def choose_tk(attn_mask):
    """Compacted key count: max unmasked count over batches, ceil to 128."""
    m = np.asarray(attn_mask)
    counts = (~m).sum(axis=0)
    tk = int(((int(counts.max()) + 127) // 128) * 128)
    return max(tk, 128)


def host_prep_core(c, tk, h, attn_mask, wq, wkv, wo, ln_g, ln_b, NH=16, DH=64):
    """Build the per-core input map (numpy) for core c."""
    T, B, D = h.shape
    F = NH * DH
    TQ = T // 2
    KC = tk // 128
    b, qh = c // 2, c % 2
    bf = ml_dtypes.bfloat16
    hb = np.roll(np.asarray(h[:, b, :], dtype=np.float32), -qh * TQ, axis=0)
    maskb = np.roll(np.asarray(attn_mask[:, b]), -qh * TQ)
    idx = np.nonzero(~maskb)[0]
    nk = idx.shape[0]
    assert nk <= tk
    idxp = np.concatenate([idx, np.zeros(tk - nk, np.int64)])
    scale = 1.0 / np.sqrt(DH)
    hbT = np.ascontiguousarray(hb.T).astype(bf)             # [D, T]
    m = {}
    m["hTq"] = np.ascontiguousarray(hbT[:, :TQ])
    m["hTk"] = np.ascontiguousarray(hbT[:, idxp])
    m["hq"] = np.ascontiguousarray(hb[:TQ])                 # [TQ, D] f32
    m["wqT"] = np.ascontiguousarray(wq.T * scale).astype(bf)
    m["wkT"] = np.ascontiguousarray(wkv[:F].T).astype(bf)
    m["wvT"] = np.ascontiguousarray(wkv[F:].T).astype(bf)
    m["woT"] = np.ascontiguousarray(wo.T).astype(bf)
    mbias = np.full(tk, NEG_BIG, np.float32)
    mbias[:nk] = 0.0
    m["maskbias"] = np.ascontiguousarray(mbias.reshape(KC, 128).T)
    m["g_rep"] = np.ascontiguousarray(
        np.broadcast_to(np.asarray(ln_g, np.float32), (128, D)))
    m["b_rep"] = np.ascontiguousarray(
        np.broadcast_to(np.asarray(ln_b, np.float32), (128, D)))
    return m

# ======================================================================
# Host-side runner: shard, compile (cached), execute on 8 cores, gather.
# ======================================================================
_NC_CACHE = {}
LAST_RESULT = None  # BassKernelResults of the most recent kernel() call


def _get_nc(T, TQ, TK, D, NH, DH):
    key = (T, TQ, TK, D, NH, DH)
    if key not in _NC_CACHE:
        _NC_CACHE[key] = build_nc(T, TQ, TK, D, NH, DH, n_cores=8, debug=False)
    return _NC_CACHE[key]


def kernel(h, attn_mask, wq, wkv, wo, ln_g, ln_b):
    """Full-input MultiHeadAttn forward on 8 NeuronCores.

    h: [T, B, D] f32; attn_mask: [T, B] bool (True = masked key);
    wq: [F, D]; wkv: [2F, D]; wo: [D, F]; ln_g/ln_b: [D].
    Returns [T, B, D] f32 = layer_norm(h + attn(h)).
    """
    from concourse.bass_utils import run_bass_kernel_spmd
    global LAST_RESULT

    h = np.asarray(h)
    attn_mask = np.asarray(attn_mask)
    wq = np.asarray(wq, np.float32)
    wkv = np.asarray(wkv, np.float32)
    wo = np.asarray(wo, np.float32)
    ln_g = np.asarray(ln_g, np.float32)
    ln_b = np.asarray(ln_b, np.float32)

    T, B, D = h.shape
    NH = 16
    DH = wq.shape[0] // NH
    assert 2 * B == 8, "sharding assumes batch 4 over 8 cores"
    TQ = T // 2
    TK = min(choose_tk(attn_mask), T)

    nc = _get_nc(T, TQ, TK, D, NH, DH)
    in_maps = [host_prep_core(c, TK, h, attn_mask, wq, wkv, wo, ln_g, ln_b,
                              NH=NH, DH=DH) for c in range(8)]
    res = run_bass_kernel_spmd(nc, in_maps, core_ids=list(range(8)))
    LAST_RESULT = res

    out = np.empty((T, B, D), np.float32)
    for c in range(8):
        b, qh = c // 2, c % 2
        out[qh * TQ:(qh + 1) * TQ, b, :] = res.results[c]["out"]
    return out


# revision 24
# speedup vs baseline: 1.1286x; 1.0870x over previous
"""Multi-head attention Bass/Tile kernel for TRN2, 8-core SPMD.

Sharding: core c handles batch b = c//2, query-half qh = c%2. The host
rotates the token axis per core so query rows sit at [0:TQ] (attention is
key-permutation invariant), and gathers the unmasked keys (mask compaction)
so K/V projection + attention only touch TK <= T key tokens.

v2: software-pipelined emission. The projection matmul chains (Q/K/V/O)
are drained a few MMs at a time between the attention matmuls so ScalarE
(softmax exp, the #2 engine) starts ~15us into the kernel instead of
~95us, and TensorE never idles waiting on exp. Other changes vs v1:
  - softmax denominator reciprocal via reciprocal_approx_fast (~5x faster
    than the iterative DVE reciprocal that cost 3.3us per (ic,hp) unit)
  - LayerNorm mean/var via DVE bn_stats/bn_aggr instead of ScalarE
    Identity/Square+accum passes; ScalarE does only exp + 2 tiny ops/tile
  - AVT normalize multiply reads PV straight from PSUM (no copy)
  - unit order (ic,hp): ic0/h0-3, ic1/h0-3, ic0/h4-7, ic1/h4-7 spreads
    the projection work evenly across the attention timeline

Matmul layouts (out = lhsT.T @ rhs, contraction on partitions):
  QT/KT [F, *] bf16 : lhsT=w*T [D,F] chunks, rhs=hT* [D,*] chunks
  V     [TK, F] bf16: lhsT=hTk chunk [D, t128], rhs=wvT [D, F]
  S^T   [j, (h0 i512 | h1 i512)] psum (2 banks): row-tiled head pair
  exp   one ACT op per j-tile: [128, 1024], bias=maskbias per-partition
  PV+den [d0:64|den 64:128, i] psum: col-tiled pairs, ones-matmul denom
  O     [t, D] psum : lhsT=AVT [f, t128], rhs=woT [f, D]
"""
from collections import deque
from types import SimpleNamespace

import numpy as np
import ml_dtypes

import concourse.bass as bass
import concourse.tile as tile
from concourse import bacc, mybir

F32 = mybir.dt.float32
BF16 = mybir.dt.bfloat16
FP8 = mybir.dt.float8e4
DR = mybir.MatmulPerfMode.DoubleRow
AF = mybir.ActivationFunctionType
ALU = mybir.AluOpType

NEG_BIG = -1.0e30
# fp8 weight pre-scale: w entries (~uniform +-0.03) are stored x16 so they
# land in e4m3's normal range instead of its 2-level subnormal range.
# Q,K both carry x16 -> scores x256; the attention SCALE (1/sqrt(dh)) and
# the 1/256 fold into the exp's free scale immediate. V,O each carry x16
# -> out-proj x256; folded into the residual-add's scalar multiplier.
WSC = 16.0


def _pin_act_tables():
    """Force every ACT func we use (Exp, Ln) to resolve to the single
    `natural_log_exp_and_others` table set, so the kernel does exactly one
    ACT_TABLE_LOAD instead of thrashing (~2.6us per switch)."""
    import concourse.hw_specs as hw_specs
    if getattr(hw_specs, "_mha_tables_pinned", False):
        return
    orig = hw_specs.get_activation_tables

    def patched(module_arch):
        tabs = orig(module_arch)
        pin = "natural_log_exp_and_others"
        if pin in tabs:
            pinned_funcs = tabs[pin]
            for name, fns in tabs.items():
                if name != pin:
                    tabs[name] = fns - pinned_funcs
        return tabs

    hw_specs.get_activation_tables = patched
    import concourse.bacc as bacc_mod
    bacc_mod.get_activation_tables = patched
    hw_specs._mha_tables_pinned = True


def _chunks(total, step):
    out = []
    off = 0
    while off < total:
        out.append((off, min(step, total - off)))
        off += step
    return out


def build_nc(T, TQ, TK, D, NH, DH, n_cores=8, debug=False):
    """Build the single-core SPMD Bass program. TK = compacted key count."""
    F = NH * DH
    DC = D // 128        # D contraction chunks
    FC = F // 128        # feature chunks (2 heads per chunk, DH=64)
    KC = TK // 128       # key tiles
    ICS = min(512, TQ)   # i-chunk size
    ICN = TQ // ICS
    FH = F // 2          # feature half (V is projected in 2 halves)
    assert DH == 64 and F % 128 == 0 and D % 128 == 0
    assert TQ % 128 == 0 and TK % 128 == 0 and ICN == 2 and FC == 8
    assert DC % 2 == 0, "fp8 DoubleRow pairs contraction chunks"

    _pin_act_tables()
    nc = bacc.Bacc("TRN2", target_bir_lowering=False, debug=debug,
                   num_devices=n_cores)

    # ---- DRAM I/O ----
    hTq_d = nc.dram_tensor("hTq", [DC * 128, TQ], FP8, kind="ExternalInput")
    hTk_d = nc.dram_tensor("hTk", [DC * 128, TK], FP8, kind="ExternalInput")
    hq_d = nc.dram_tensor("hq", [TQ, D], F32, kind="ExternalInput")
    wqT_d = nc.dram_tensor("wqT", [DC * 128, F], FP8, kind="ExternalInput")
    wkT_d = nc.dram_tensor("wkT", [DC * 128, F], FP8, kind="ExternalInput")
    wvT_d = nc.dram_tensor("wvT", [DC * 128, F], FP8, kind="ExternalInput")
    woT_d = nc.dram_tensor("woT", [FC * 128, D], FP8, kind="ExternalInput")
    mb_d = nc.dram_tensor("maskbias", [128, KC], F32, kind="ExternalInput")
    g_d = nc.dram_tensor("g_rep", [128, D], F32, kind="ExternalInput")
    b_d = nc.dram_tensor("b_rep", [128, D], F32, kind="ExternalInput")
    out_d = nc.dram_tensor("out", [TQ, D], F32, kind="ExternalOutput")

    with tile.TileContext(nc) as tc:
        with (
            tc.tile_pool(name="big", bufs=1) as big,
            tc.tile_pool(name="ep", bufs=4) as ep,
            tc.tile_pool(name="hqp", bufs=2) as hqp,
            tc.tile_pool(name="lnp", bufs=2) as lnp,
            tc.tile_pool(name="pvs", bufs=1) as pvs,
            tc.tile_pool(name="statp", bufs=2) as statp,
            tc.tile_pool(name="psS", bufs=2, space="PSUM") as psS,
            tc.tile_pool(name="psPV", bufs=2, space="PSUM") as psPV,
            tc.tile_pool(name="psP", bufs=2, space="PSUM") as psP,
        ):
            # ---- persistent SBUF tiles ----
            # fp8 operand tiles are 3D [128, chunk, n]: DoubleRow matmuls
            # slice [:, c:c+2, :] to contract 256 rows per instruction.
            hTq = big.tile([128, DC, TQ], FP8, tag="htq")
            hTk = big.tile([128, DC, TK], FP8, tag="htk")
            wqT = big.tile([128, DC, F], FP8, tag="wq")
            wkT = big.tile([128, DC, F], FP8, tag="wk")
            wvT = big.tile([128, DC, F], FP8, tag="wv")
            woT = big.tile([128, FC, D], FP8, tag="wo")
            QT = big.tile([128, FC * TQ], BF16, tag="qt")
            KT = big.tile([128, FC * TK], BF16, tag="kt")
            V = big.tile([128, KC * F], BF16, tag="v")
            AVT = big.tile([128, FC, TQ], FP8, tag="avt")
            g_re = big.tile([128, D], F32, tag="g")
            b_re = big.tile([128, D], F32, tag="b")
            ones = big.tile([128, 64], BF16, tag="ones")
            mb = big.tile([128, KC], F32, tag="mb")
            eps_t = big.tile([128, 1], F32, tag="eps")

            nc.vector.memset(ones[:], 1.0)
            nc.vector.memset(eps_t[:], 1e-5)
            nc.sync.dma_start(mb[:], mb_d[:])

            # ---- DMA in consumption order ----
            def sl(t3, c, a, b):
                return t3[:, c:c + 1, a:b].squeeze(1)

            # First wave: what the prologue + first unit touch.
            for dc in range(DC):
                nc.sync.dma_start(sl(wqT, dc, 0, F),
                                  wqT_d[dc * 128:(dc + 1) * 128, :])
            for dc in range(DC):  # hTq i0 half (Q(0,i0) rhs)
                nc.sync.dma_start(sl(hTq, dc, 0, ICS),
                                  hTq_d[dc * 128:(dc + 1) * 128, 0:ICS])
            for dc in range(DC):  # wk cols of hp0
                nc.sync.dma_start(sl(wkT, dc, 0, 128),
                                  wkT_d[dc * 128:(dc + 1) * 128, 0:128])
            hTk_half = min(512, TK)
            for dc in range(DC):  # hTk first 512 keys
                nc.sync.dma_start(sl(hTk, dc, 0, hTk_half),
                                  hTk_d[dc * 128:(dc + 1) * 128, 0:hTk_half])
            for dc in range(DC):  # wv first feature half
                nc.sync.dma_start(sl(wvT, dc, 0, FH),
                                  wvT_d[dc * 128:(dc + 1) * 128, 0:FH])
            # Second wave: the rest of the preload.
            for dc in range(DC):
                nc.sync.dma_start(sl(hTq, dc, ICS, TQ),
                                  hTq_d[dc * 128:(dc + 1) * 128, ICS:TQ])
                nc.sync.dma_start(sl(hTk, dc, hTk_half, TK),
                                  hTk_d[dc * 128:(dc + 1) * 128, hTk_half:TK])
                nc.sync.dma_start(sl(wkT, dc, 128, F),
                                  wkT_d[dc * 128:(dc + 1) * 128, 128:F])
                nc.sync.dma_start(sl(wvT, dc, FH, F),
                                  wvT_d[dc * 128:(dc + 1) * 128, FH:F])

            # ================= projection chain machinery =================
            # All projections run fp8 DoubleRow: 4 matmuls of K=256 each.
            def q_chain(hp, icx):
                io = icx * ICS
                ch = SimpleNamespace(ps=None)

                def mm(dc):
                    def f():
                        if ch.ps is None:
                            ch.ps = psP.tile([128, ICS], F32, tag="P",
                                             name="qps")
                        nc.tensor.matmul(
                            ch.ps[:],
                            wqT[:, dc:dc + 2, hp * 128:(hp + 1) * 128],
                            hTq[:, dc:dc + 2, io: io + ICS],
                            start=(dc == 0), stop=(dc == DC - 2),
                            perf_mode=DR, skip_group_check=True)
                    return f

                def fin():
                    nc.vector.tensor_copy(
                        QT[:, hp * TQ + io: hp * TQ + io + ICS], ch.ps[:])
                return SimpleNamespace(mms=deque(mm(dc) for dc in range(0, DC, 2)),
                                       fin=fin)

            def k_chain(hp, c0, cn):
                ch = SimpleNamespace(ps=None)

                def mm(dc):
                    def f():
                        if ch.ps is None:
                            ch.ps = psP.tile([128, cn], F32, tag="P",
                                             name="kps")
                        nc.tensor.matmul(
                            ch.ps[:],
                            wkT[:, dc:dc + 2, hp * 128:(hp + 1) * 128],
                            hTk[:, dc:dc + 2, c0: c0 + cn],
                            start=(dc == 0), stop=(dc == DC - 2),
                            perf_mode=DR, skip_group_check=True)
                    return f

                def fin():
                    nc.vector.tensor_copy(
                        KT[:, hp * TK + c0: hp * TK + c0 + cn], ch.ps[:])
                return SimpleNamespace(mms=deque(mm(dc) for dc in range(0, DC, 2)),
                                       fin=fin)

            def v_chain(jc, fh):
                f0 = fh * FH
                ch = SimpleNamespace(ps=None)

                def mm(dc):
                    def f():
                        if ch.ps is None:
                            ch.ps = psP.tile([128, FH], F32, tag="P",
                                             name="vps")
                        nc.tensor.matmul(
                            ch.ps[:],
                            hTk[:, dc:dc + 2, jc * 128:(jc + 1) * 128],
                            wvT[:, dc:dc + 2, f0: f0 + FH],
                            start=(dc == 0), stop=(dc == DC - 2),
                            perf_mode=DR, skip_group_check=True)
                    return f

                def fin():
                    nc.vector.tensor_copy(
                        V[:, jc * F + f0: jc * F + f0 + FH], ch.ps[:])
                return SimpleNamespace(mms=deque(mm(dc) for dc in range(0, DC, 2)),
                                       fin=fin)

            # O-proj + residual + LayerNorm, per t-tile of 128 queries.
            tt_state = {}

            def ln_tail(tt, st):
                st6 = statp.tile([128, 12], F32, tag="st6")
                nc.vector.bn_stats(st6[:, 0:6], st.x[:, 0:512])
                nc.vector.bn_stats(st6[:, 6:12], st.x[:, 512:1024])
                mv = statp.tile([128, 2], F32, tag="mv")
                nc.vector.bn_aggr(mv[:], st6[:])
                lnv = statp.tile([128, 1], F32, tag="lnv")
                rstd = statp.tile([128, 1], F32, tag="rstd")
                # rstd = exp(-0.5*ln(var+eps)); Ln+Exp share one ACT table set
                nc.scalar.activation(lnv[:], mv[:, 1:2], AF.Ln, bias=eps_t[:])
                nc.scalar.activation(rstd[:], lnv[:], AF.Exp, scale=-0.5)
                xc = lnp.tile([128, D], F32, tag="xc", bufs=1)
                nc.vector.scalar_tensor_tensor(
                    xc[:], st.x[:], mv[:, 0:1], g_re[:],
                    op0=ALU.subtract, op1=ALU.mult)
                # write into the dead hqt tile, then DMA out
                nc.vector.scalar_tensor_tensor(
                    st.hqt[:], xc[:], rstd[:], b_re[:],
                    op0=ALU.mult, op1=ALU.add)
                nc.sync.dma_start(out_d[tt * 128:(tt + 1) * 128, :], st.hqt[:])

            def o_chain(tt, dci):
                d0 = dci * 512
                ch = SimpleNamespace(ps=None)

                def mm(fc):
                    def f():
                        if ch.ps is None:
                            if tt not in tt_state:
                                hqt = hqp.tile([128, D], F32, tag="hq")
                                nc.sync.dma_start(
                                    hqt[:], hq_d[tt * 128:(tt + 1) * 128, :])
                                x = lnp.tile([128, D], F32, tag="x")
                                tt_state[tt] = SimpleNamespace(
                                    hqt=hqt, x=x, done=0)
                            ch.ps = psP.tile([128, 512], F32, tag="P",
                                             name="ops")
                        nc.tensor.matmul(
                            ch.ps[:],
                            AVT[:, fc:fc + 2, tt * 128:(tt + 1) * 128],
                            woT[:, fc:fc + 2, d0: d0 + 512],
                            start=(fc == 0), stop=(fc == FC - 2),
                            perf_mode=DR, skip_group_check=True)
                    return f

                def fin():
                    st = tt_state[tt]
                    # residual add; 1/WSC^2 undoes the V and O weight scales
                    nc.vector.scalar_tensor_tensor(
                        st.x[:, d0:d0 + 512], ch.ps[:], 1.0 / (WSC * WSC),
                        st.hqt[:, d0:d0 + 512], op0=ALU.mult, op1=ALU.add)
                    st.done += 1
                    if st.done == 2:
                        ln_tail(tt, st)
                return SimpleNamespace(mms=deque(mm(fc) for fc in range(0, FC, 2)),
                                       fin=fin)

            # ---- build the drain queue in deadline order ----
            # Every chain carries a `deadline` (unit_idx, loop_iter): its data
            # is consumed by the attention emission at that position, so it
            # must be FULLY emitted before then (Tile tracks deps in program
            # order — a read emitted before its producing write races).
            # Chains whose matmuls depend on the attention pipeline itself
            # (the O-proj chains reading AVT) carry a `gate`: pump() must not
            # emit them until that many units are fully emitted, else the
            # in-order PE queue deadlocks on its own later instructions.
            # unit order (ic, hp); index in this list is the deadline key.
            units = [(0, 0), (0, 1), (0, 2), (0, 3),
                     (1, 0), (1, 1), (1, 2), (1, 3),
                     (0, 4), (0, 5), (0, 6), (0, 7),
                     (1, 4), (1, 5), (1, 6), (1, 7)]
            uidx_of = {u: i for i, u in enumerate(units)}
            LAG = 2
            FAR = (99, 0)

            kchunks = _chunks(TK, 512)
            queue = deque()
            units_done = SimpleNamespace(n=0)
            qmm = SimpleNamespace(n=0)

            def push(ch, deadline=FAR, gate=0):
                ch.gate = gate
                ch.deadline = deadline
                qmm.n += len(ch.mms)
                queue.append(ch)

            def k_dl(hp, ci):
                return (uidx_of[(0, hp)], (kchunks[ci][0]) // 128)

            def q_dl(hp, icx):
                return (uidx_of[(icx, hp)], 0)

            def v_dl(jc, fh):
                return (uidx_of[(0, 0 if fh == 0 else 4)], jc + LAG)

            # u1 needs: V(4..KC-1, fh0), KT(0,c1)
            fh0_rest = list(range(4, KC))
            if fh0_rest:
                push(v_chain(fh0_rest[0], 0), v_dl(fh0_rest[0], 0))
            for ci in range(1, len(kchunks)):
                push(k_chain(0, *kchunks[ci]), k_dl(0, ci))
            for jc in fh0_rest[1:]:
                push(v_chain(jc, 0), v_dl(jc, 0))
            # u2..u4: KT/QT for hp 1..3 (+ first V fh1 chains)
            for hp in (1, 2, 3):
                push(k_chain(hp, *kchunks[0]), k_dl(hp, 0))
                push(q_chain(hp, 0), q_dl(hp, 0))
                for ci in range(1, len(kchunks)):
                    push(k_chain(hp, *kchunks[ci]), k_dl(hp, ci))
                if hp >= 2:
                    push(v_chain(hp - 2, 1), v_dl(hp - 2, 1))
            # u5..u8: QT(0..3, i1), rest of V fh1, KT/QT hp4
            push(q_chain(0, 1), q_dl(0, 1))
            push(v_chain(2, 1), v_dl(2, 1))
            push(q_chain(1, 1), q_dl(1, 1))
            push(v_chain(3, 1), v_dl(3, 1))
            push(q_chain(2, 1), q_dl(2, 1))
            push(v_chain(4 % KC, 1), v_dl(4 % KC, 1))
            push(k_chain(4, *kchunks[0]), k_dl(4, 0))
            push(q_chain(4, 0), q_dl(4, 0))
            push(q_chain(3, 1), q_dl(3, 1))
            for ci in range(1, len(kchunks)):
                push(k_chain(4, *kchunks[ci]), k_dl(4, ci))
            for jc in range(5, KC):
                push(v_chain(jc, 1), v_dl(jc, 1))
            # u9..u12: KT/QT hp 5..7
            for hp in (5, 6, 7):
                push(k_chain(hp, *kchunks[0]), k_dl(hp, 0))
                push(q_chain(hp, 0), q_dl(hp, 0))
                for ci in range(1, len(kchunks)):
                    push(k_chain(hp, *kchunks[ci]), k_dl(hp, ci))
            push(q_chain(4, 1), q_dl(4, 1))
            # u13..u16: QT(5..7, i1) + O(ic0) (gated on AVT(ic0) complete)
            for hp in (5, 6, 7):
                push(q_chain(hp, 1), q_dl(hp, 1))
            for tt in range(0, ICS // 128):
                push(o_chain(tt, 0), FAR, gate=12)
                push(o_chain(tt, 1), FAR, gate=12)
            # tail: O(ic1) (gated on AVT(ic1) complete)
            for tt in range(ICS // 128, TQ // 128):
                push(o_chain(tt, 0), FAR, gate=16)
                push(o_chain(tt, 1), FAR, gate=16)

            def run_rest(ch):
                while ch.mms:
                    ch.mms.popleft()()
                    qmm.n -= 1
                ch.fin()

            def drain_due(pos):
                """Fully emit every queued chain whose deadline has arrived."""
                due = [ch for ch in queue if ch.deadline <= pos]
                for ch in due:
                    queue.remove(ch)
                    run_rest(ch)

            def pump(n):
                while n > 0 and queue:
                    ch = queue[0]
                    if ch.gate > units_done.n:
                        return
                    ch.mms.popleft()()
                    qmm.n -= 1
                    n -= 1
                    if not ch.mms:
                        ch.fin()
                        queue.popleft()

            # ---- prologue projections (emitted whole, never queued) ----
            def run_chain(ch):
                while ch.mms:
                    ch.mms.popleft()()
                ch.fin()

            run_chain(q_chain(0, 0))
            run_chain(k_chain(0, *kchunks[0]))
            for jc in range(min(4, KC)):
                run_chain(v_chain(jc, 0))

            # late loads (overlap with attention)
            for fc_ in range(FC):
                nc.sync.dma_start(sl(woT, fc_, 0, D),
                                  woT_d[fc_ * 128:(fc_ + 1) * 128, :])
            nc.sync.dma_start(g_re[:], g_d[:])
            nc.sync.dma_start(b_re[:], b_d[:])

            # ================= attention units =================
            slots = SimpleNamespace(n=len(units) * KC)

            for uidx, (icx, hp) in enumerate(units):
                io = icx * ICS
                h0, h1 = 2 * hp, 2 * hp + 1
                pvP = psPV.tile([128, ICS], F32, tag="pv", name="pvP")
                pvD = psPV.tile([128, ICS], F32, tag="pv", name="pvD")
                e_tiles = {}

                def emit_S(jc):
                    s = psS.tile([128, 2 * ICS], F32, tag="S", name="s")
                    nc.tensor.matmul(
                        s[:, 0:ICS],
                        KT[0:64, hp * TK + jc * 128: hp * TK + (jc + 1) * 128],
                        QT[0:64, hp * TQ + io: hp * TQ + io + ICS],
                        start=True, stop=True, tile_position=(0, 0),
                        skip_group_check=True)
                    nc.tensor.matmul(
                        s[:, ICS:2 * ICS],
                        KT[64:128, hp * TK + jc * 128: hp * TK + (jc + 1) * 128],
                        QT[64:128, hp * TQ + io: hp * TQ + io + ICS],
                        start=True, stop=True, tile_position=(64, 0),
                        skip_group_check=True)
                    e = ep.tile([128, 2 * ICS], BF16, tag="e", name="e")
                    # scale folds attention 1/sqrt(dh) and the x16 fp8
                    # pre-scales of wq and wk back out of the raw scores
                    nc.scalar.activation(e[:], s[:], AF.Exp,
                                         bias=mb[:, jc:jc + 1],
                                         scale=1.0 / (WSC * WSC * DH ** 0.5))
                    e_tiles[jc] = e

                def emit_PV(jc):
                    e = e_tiles.pop(jc)
                    st, sp = (jc == 0), (jc == KC - 1)
                    nc.tensor.matmul(
                        pvP[0:64, :],
                        V[:, jc * F + h0 * DH: jc * F + (h0 + 1) * DH],
                        e[:, 0:ICS], start=st, stop=sp,
                        tile_position=(0, 0), skip_group_check=True)
                    nc.tensor.matmul(
                        pvP[64:128, :],
                        V[:, jc * F + h1 * DH: jc * F + (h1 + 1) * DH],
                        e[:, ICS:2 * ICS], start=st, stop=sp,
                        tile_position=(0, 64), skip_group_check=True)
                    nc.tensor.matmul(
                        pvD[0:64, :], ones[:, 0:64],
                        e[:, 0:ICS], start=st, stop=sp,
                        tile_position=(0, 0), skip_group_check=True)
                    nc.tensor.matmul(
                        pvD[64:128, :], ones[:, 0:64],
                        e[:, ICS:2 * ICS], start=st, stop=sp,
                        tile_position=(0, 64), skip_group_check=True)

                for jc in range(KC + LAG):
                    # force-emit any chain whose consumer is imminent
                    drain_due((uidx, jc))
                    if jc < KC:
                        emit_S(jc)
                        # pace the projection queue across the whole kernel
                        rate = -(-qmm.n // max(slots.n, 1))
                        pump(min(rate, 6))
                        slots.n -= 1
                    else:
                        pump(2)
                    if jc >= LAG:
                        emit_PV(jc - LAG)

                # normalize -> AVT: approx-reciprocal of the denominator,
                # multiply PV (still in PSUM) by it.
                pvDc = pvs.tile([128, ICS], F32, tag="pd")
                nc.vector.tensor_copy(pvDc[:], pvD[:])
                rec = pvs.tile([128, ICS], F32, tag="rc")
                nc.vector.reciprocal_approx_fast(rec[:], pvDc[:])
                nc.vector.tensor_tensor(
                    AVT[:, hp:hp + 1, io: io + ICS].squeeze(1),
                    pvP[:], rec[:], op=ALU.mult)
                units_done.n += 1

            # drain whatever projection work remains (O(ic1) mostly)
            pump(10 ** 9)

    nc.compile()
    return nc


def choose_tk(attn_mask):
    """Compacted key count: max unmasked count over batches, ceil to 128."""
    m = np.asarray(attn_mask)
    counts = (~m).sum(axis=0)
    tk = int(((int(counts.max()) + 127) // 128) * 128)
    return max(tk, 128)


def host_prep_core(c, tk, h, attn_mask, wq, wkv, wo, ln_g, ln_b, NH=16, DH=64):
    """Build the per-core input map (numpy) for core c."""
    T, B, D = h.shape
    F = NH * DH
    TQ = T // 2
    KC = tk // 128
    b, qh = c // 2, c % 2
    f8 = ml_dtypes.float8_e4m3
    hb = np.roll(np.asarray(h[:, b, :], dtype=np.float32), -qh * TQ, axis=0)
    maskb = np.roll(np.asarray(attn_mask[:, b]), -qh * TQ)
    idx = np.nonzero(~maskb)[0]
    nk = idx.shape[0]
    assert nk <= tk
    idxp = np.concatenate([idx, np.zeros(tk - nk, np.int64)])
    hbT = np.ascontiguousarray(hb.T).astype(f8)             # [D, T]
    m = {}
    m["hTq"] = np.ascontiguousarray(hbT[:, :TQ])
    m["hTk"] = np.ascontiguousarray(hbT[:, idxp])
    m["hq"] = np.ascontiguousarray(hb[:TQ])                 # [TQ, D] f32
    # weights stored x16 in fp8 (see WSC); compensated in-kernel
    m["wqT"] = np.ascontiguousarray(wq.T * WSC).astype(f8)
    m["wkT"] = np.ascontiguousarray(wkv[:F].T * WSC).astype(f8)
    m["wvT"] = np.ascontiguousarray(wkv[F:].T * WSC).astype(f8)
    m["woT"] = np.ascontiguousarray(wo.T * WSC).astype(f8)
    mbias = np.full(tk, NEG_BIG, np.float32)
    mbias[:nk] = 0.0
    m["maskbias"] = np.ascontiguousarray(mbias.reshape(KC, 128).T)
    m["g_rep"] = np.ascontiguousarray(
        np.broadcast_to(np.asarray(ln_g, np.float32), (128, D)))
    m["b_rep"] = np.ascontiguousarray(
        np.broadcast_to(np.asarray(ln_b, np.float32), (128, D)))
    return m

# ======================================================================
# Host-side runner: shard, compile (cached), execute on 8 cores, gather.
# ======================================================================
_NC_CACHE = {}
LAST_RESULT = None  # BassKernelResults of the most recent kernel() call


def _get_nc(T, TQ, TK, D, NH, DH):
    key = (T, TQ, TK, D, NH, DH)
    if key not in _NC_CACHE:
        _NC_CACHE[key] = build_nc(T, TQ, TK, D, NH, DH, n_cores=8, debug=False)
    return _NC_CACHE[key]


def kernel(h, attn_mask, wq, wkv, wo, ln_g, ln_b):
    """Full-input MultiHeadAttn forward on 8 NeuronCores.

    h: [T, B, D] f32; attn_mask: [T, B] bool (True = masked key);
    wq: [F, D]; wkv: [2F, D]; wo: [D, F]; ln_g/ln_b: [D].
    Returns [T, B, D] f32 = layer_norm(h + attn(h)).
    """
    from concourse.bass_utils import run_bass_kernel_spmd
    global LAST_RESULT

    h = np.asarray(h)
    attn_mask = np.asarray(attn_mask)
    wq = np.asarray(wq, np.float32)
    wkv = np.asarray(wkv, np.float32)
    wo = np.asarray(wo, np.float32)
    ln_g = np.asarray(ln_g, np.float32)
    ln_b = np.asarray(ln_b, np.float32)

    T, B, D = h.shape
    NH = 16
    DH = wq.shape[0] // NH
    assert 2 * B == 8, "sharding assumes batch 4 over 8 cores"
    TQ = T // 2
    TK = min(choose_tk(attn_mask), T)

    nc = _get_nc(T, TQ, TK, D, NH, DH)
    in_maps = [host_prep_core(c, TK, h, attn_mask, wq, wkv, wo, ln_g, ln_b,
                              NH=NH, DH=DH) for c in range(8)]
    res = run_bass_kernel_spmd(nc, in_maps, core_ids=list(range(8)))
    LAST_RESULT = res

    out = np.empty((T, B, D), np.float32)
    for c in range(8):
        b, qh = c // 2, c % 2
        out[qh * TQ:(qh + 1) * TQ, b, :] = res.results[c]["out"]
    return out


# revision 37
# speedup vs baseline: 1.3276x; 1.1763x over previous
"""Multi-head attention Bass/Tile kernel for TRN2, 8-core SPMD.

Sharding: core c handles batch b = c//2, query-half qh = c%2. The host
rotates the token axis per core so query rows sit at [0:TQ] (attention is
key-permutation invariant), and gathers the unmasked keys (mask compaction)
so K/V projection + attention only touch TK <= T key tokens.

v2: software-pipelined emission. The projection matmul chains (Q/K/V/O)
are drained a few MMs at a time between the attention matmuls so ScalarE
(softmax exp, the #2 engine) starts ~15us into the kernel instead of
~95us, and TensorE never idles waiting on exp. Other changes vs v1:
  - softmax denominator reciprocal via reciprocal_approx_fast (~5x faster
    than the iterative DVE reciprocal that cost 3.3us per (ic,hp) unit)
  - LayerNorm mean/var via DVE bn_stats/bn_aggr instead of ScalarE
    Identity/Square+accum passes; ScalarE does only exp + 2 tiny ops/tile
  - AVT normalize multiply reads PV straight from PSUM (no copy)
  - unit order (ic,hp): ic0/h0-3, ic1/h0-3, ic0/h4-7, ic1/h4-7 spreads
    the projection work evenly across the attention timeline

Matmul layouts (out = lhsT.T @ rhs, contraction on partitions):
  QT/KT [F, *] bf16 : lhsT=w*T [D,F] chunks, rhs=hT* [D,*] chunks
  V     [TK, F] bf16: lhsT=hTk chunk [D, t128], rhs=wvT [D, F]
  S^T   [j, (h0 i512 | h1 i512)] psum (2 banks): row-tiled head pair
  exp   one ACT op per j-tile: [128, 1024], bias=maskbias per-partition
  PV+den [d0:64|den 64:128, i] psum: col-tiled pairs, ones-matmul denom
  O     [t, D] psum : lhsT=AVT [f, t128], rhs=woT [f, D]
"""
from collections import deque
from types import SimpleNamespace

import numpy as np
import ml_dtypes

import concourse.bass as bass
import concourse.tile as tile
from concourse import bacc, mybir

F32 = mybir.dt.float32
BF16 = mybir.dt.bfloat16
FP8 = mybir.dt.float8e4
FP8E5 = mybir.dt.float8e5
DR = mybir.MatmulPerfMode.DoubleRow
AF = mybir.ActivationFunctionType
ALU = mybir.AluOpType

NEG_BIG = -1.0e30
# fp8 weight pre-scale: w entries (~uniform +-0.03) are stored x16 so they
# land in e4m3's normal range instead of its 2-level subnormal range.
# Q,K both carry x16 -> scores x256; the attention SCALE (1/sqrt(dh)) and
# the 1/256 fold into the exp's free scale immediate. V,O each carry x16
# -> out-proj x256; folded into the residual-add's scalar multiplier.
WSC = 16.0


def _pin_act_tables():
    """Force every ACT func we use (Exp, Ln) to resolve to the single
    `natural_log_exp_and_others` table set, so the kernel does exactly one
    ACT_TABLE_LOAD instead of thrashing (~2.6us per switch)."""
    import concourse.hw_specs as hw_specs
    if getattr(hw_specs, "_mha_tables_pinned", False):
        return
    orig = hw_specs.get_activation_tables

    def patched(module_arch):
        tabs = orig(module_arch)
        pin = "natural_log_exp_and_others"
        if pin in tabs:
            pinned_funcs = tabs[pin]
            for name, fns in tabs.items():
                if name != pin:
                    tabs[name] = fns - pinned_funcs
        return tabs

    hw_specs.get_activation_tables = patched
    import concourse.bacc as bacc_mod
    bacc_mod.get_activation_tables = patched
    hw_specs._mha_tables_pinned = True


def _chunks(total, step):
    out = []
    off = 0
    while off < total:
        out.append((off, min(step, total - off)))
        off += step
    return out


def build_nc(T, TQ, TK, D, NH, DH, n_cores=8, debug=False):
    """Build the single-core SPMD Bass program. TK = compacted key count."""
    F = NH * DH
    DC = D // 128        # D contraction chunks
    FC = F // 128        # feature chunks (2 heads per chunk, DH=64)
    KC = TK // 128       # key tiles
    ICS = min(512, TQ)   # i-chunk size
    ICN = TQ // ICS
    FH = F // 2          # feature half (V is projected in 2 halves)
    assert DH == 64 and F % 128 == 0 and D % 128 == 0
    assert TQ % 128 == 0 and TK % 128 == 0 and ICN == 2 and FC == 8
    assert DC % 2 == 0, "fp8 DoubleRow pairs contraction chunks"
    assert KC % 2 == 0, "PV DoubleRow pairs key tiles (choose_tk rounds to 256)"
    KC2 = KC // 2

    _pin_act_tables()
    nc = bacc.Bacc("TRN2", target_bir_lowering=False, debug=debug,
                   num_devices=n_cores)

    # ---- DRAM I/O ----
    hTq_d = nc.dram_tensor("hTq", [DC * 128, TQ], FP8, kind="ExternalInput")
    hTk_d = nc.dram_tensor("hTk", [DC * 128, TK], FP8, kind="ExternalInput")
    hq_d = nc.dram_tensor("hq", [TQ, D], F32, kind="ExternalInput")
    wqT_d = nc.dram_tensor("wqT", [DC * 128, F], FP8, kind="ExternalInput")
    wkT_d = nc.dram_tensor("wkT", [DC * 128, F], FP8, kind="ExternalInput")
    wvT_d = nc.dram_tensor("wvT", [DC * 128, F], FP8, kind="ExternalInput")
    woT_d = nc.dram_tensor("woT", [FC * 128, D], FP8, kind="ExternalInput")
    mb_d = nc.dram_tensor("maskbias", [128, KC], F32, kind="ExternalInput")
    g_d = nc.dram_tensor("g_rep", [128, D], F32, kind="ExternalInput")
    b_d = nc.dram_tensor("b_rep", [128, D], F32, kind="ExternalInput")
    out_d = nc.dram_tensor("out", [TQ, D], F32, kind="ExternalOutput")

    with tile.TileContext(nc) as tc:
        with (
            tc.tile_pool(name="big", bufs=1) as big,
            tc.tile_pool(name="ep", bufs=4) as ep,
            tc.tile_pool(name="hqp", bufs=5) as hqp,
            tc.tile_pool(name="lnp", bufs=5) as lnp,
            tc.tile_pool(name="pvs", bufs=1) as pvs,
            tc.tile_pool(name="statp", bufs=2) as statp,
            tc.tile_pool(name="psS", bufs=2, space="PSUM") as psS,
            tc.tile_pool(name="psPV", bufs=2, space="PSUM") as psPV,
            tc.tile_pool(name="psP", bufs=2, space="PSUM") as psP,
        ):
            # ---- persistent SBUF tiles ----
            # fp8 operand tiles are 3D [128, chunk, n]: DoubleRow matmuls
            # slice [:, c:c+2, :] to contract 256 rows per instruction.
            hTq = big.tile([128, DC, TQ], FP8, tag="htq")
            hTk = big.tile([128, DC, TK], FP8, tag="htk")
            wqT = big.tile([128, DC, F], FP8, tag="wq")
            wkT = big.tile([128, DC, F], FP8, tag="wk")
            wvT = big.tile([128, DC, F], FP8, tag="wv")
            woT = big.tile([128, FC, D], FP8, tag="wo")
            QT = big.tile([128, FC * TQ], BF16, tag="qt")
            KT = big.tile([128, FC * TK], BF16, tag="kt")
            # V8: fp8 PV weights, DoubleRow layout [key-pair, kh, head-block].
            # Head block hh is 128 wide: even hh = [V_hh | ones], odd hh =
            # [ones | V_hh], so the PV matmul's spare 64 output partitions
            # compute the softmax denominator for free, and the two heads'
            # PV/den land on complementary partition halves.
            V8 = big.tile([128, KC2, 2, NH // 2, 256], FP8, tag="v")
            AVT = big.tile([128, FC, TQ], FP8, tag="avt")
            g_re = big.tile([128, D], F32, tag="g")
            b_re = big.tile([128, D], F32, tag="b")
            mb = big.tile([128, KC], F32, tag="mb")
            eps_t = big.tile([128, 1], F32, tag="eps")

            nc.vector.memset(V8[:], 1.0)  # ones blocks; V blocks overwritten
            nc.vector.memset(eps_t[:], 1e-5)
            nc.sync.dma_start(mb[:], mb_d[:])

            # ---- DMA in consumption order ----
            def sl(t3, c, a, b):
                return t3[:, c:c + 1, a:b].squeeze(1)

            # First wave: everything the prologue + unit 1 touch (fp8 makes
            # this ~4MB: wq, hTq-i0, wk, hTk in full, wv first half).
            for dc in range(DC):
                nc.sync.dma_start(sl(wqT, dc, 0, F),
                                  wqT_d[dc * 128:(dc + 1) * 128, :])
            for dc in range(DC):  # hTq i0 half (Q(0,i0) rhs)
                nc.sync.dma_start(sl(hTq, dc, 0, ICS),
                                  hTq_d[dc * 128:(dc + 1) * 128, 0:ICS])
            for dc in range(DC):
                nc.sync.dma_start(sl(wkT, dc, 0, F),
                                  wkT_d[dc * 128:(dc + 1) * 128, :])
            for dc in range(DC):
                nc.sync.dma_start(sl(hTk, dc, 0, TK),
                                  hTk_d[dc * 128:(dc + 1) * 128, :])
            for dc in range(DC):  # wv first feature half
                nc.sync.dma_start(sl(wvT, dc, 0, FH),
                                  wvT_d[dc * 128:(dc + 1) * 128, 0:FH])
            # Second wave: the rest of the preload.
            for dc in range(DC):
                nc.sync.dma_start(sl(hTq, dc, ICS, TQ),
                                  hTq_d[dc * 128:(dc + 1) * 128, ICS:TQ])
                nc.sync.dma_start(sl(wvT, dc, FH, F),
                                  wvT_d[dc * 128:(dc + 1) * 128, FH:F])

            # ================= projection chain machinery =================
            # All projections run fp8 DoubleRow: 4 matmuls of K=256 each.
            def q_chain(hp, icx):
                io = icx * ICS
                ch = SimpleNamespace(ps=None)

                def mm(dc):
                    def f():
                        if ch.ps is None:
                            ch.ps = psP.tile([128, ICS], F32, tag="P",
                                             name="qps")
                        nc.tensor.matmul(
                            ch.ps[:],
                            wqT[:, dc:dc + 2, hp * 128:(hp + 1) * 128],
                            hTq[:, dc:dc + 2, io: io + ICS],
                            start=(dc == 0), stop=(dc == DC - 2),
                            perf_mode=DR, skip_group_check=True)
                    return f

                def fin():
                    nc.vector.tensor_copy(
                        QT[:, hp * TQ + io: hp * TQ + io + ICS], ch.ps[:])
                return SimpleNamespace(mms=deque(mm(dc) for dc in range(0, DC, 2)),
                                       fin=fin)

            def k_chain(hp, c0, cn):
                ch = SimpleNamespace(ps=None)

                def mm(dc):
                    def f():
                        if ch.ps is None:
                            ch.ps = psP.tile([128, cn], F32, tag="P",
                                             name="kps")
                        nc.tensor.matmul(
                            ch.ps[:],
                            wkT[:, dc:dc + 2, hp * 128:(hp + 1) * 128],
                            hTk[:, dc:dc + 2, c0: c0 + cn],
                            start=(dc == 0), stop=(dc == DC - 2),
                            perf_mode=DR, skip_group_check=True)
                    return f

                def fin():
                    nc.vector.tensor_copy(
                        KT[:, hp * TK + c0: hp * TK + c0 + cn], ch.ps[:])
                return SimpleNamespace(mms=deque(mm(dc) for dc in range(0, DC, 2)),
                                       fin=fin)

            def v_chain(jc, fh):
                f0 = fh * FH
                ch = SimpleNamespace(ps=None)

                def mm(dc):
                    def f():
                        if ch.ps is None:
                            # [head-pair, parity, 64] view of the 512 f-cols
                            ch.ps = psP.tile([128, 4, 2, 64], F32, tag="P",
                                             name="vps")
                        nc.tensor.matmul(
                            ch.ps[:, :, :, :],
                            hTk[:, dc:dc + 2, jc * 128:(jc + 1) * 128],
                            wvT[:, dc:dc + 2, f0: f0 + FH],
                            start=(dc == 0), stop=(dc == DC - 2),
                            perf_mode=DR, skip_group_check=True)
                    return f

                def fin():
                    # scatter the 8 heads of this half into V8's alternating
                    # [V|ones] / [ones|V] 128-blocks (even: V at +0, odd: +64
                    # of the odd half-block, i.e. +192 within the 256 pair)
                    p2, kh, h4 = jc // 2, jc % 2, fh * 4
                    dste = V8[:, p2:p2 + 1, kh:kh + 1, h4:h4 + 4, 0:64]
                    nc.vector.tensor_copy(
                        dste.squeeze(1).squeeze(1),
                        ch.ps[:, :, 0:1, :].squeeze(2))
                    dsto = V8[:, p2:p2 + 1, kh:kh + 1, h4:h4 + 4, 192:256]
                    nc.vector.tensor_copy(
                        dsto.squeeze(1).squeeze(1),
                        ch.ps[:, :, 1:2, :].squeeze(2))
                return SimpleNamespace(mms=deque(mm(dc) for dc in range(0, DC, 2)),
                                       fin=fin)

            # O-proj + residual + LayerNorm, per t-tile of 128 queries.
            tt_state = {}

            # LN emissions are deferred while the attention units are still
            # being emitted: the in-order DVE queue otherwise buries the last
            # units' AVT-normalize (which gates the tail O matmuls on PE)
            # behind ~15us of LayerNorm work. Deferred LNs run while the PE
            # chews the tail O chains.
            ln_defer = SimpleNamespace(on=True, q=[])

            def ln_tail(tt, st):
                if ln_defer.on:
                    ln_defer.q.append((tt, st))
                    return
                st6 = statp.tile([128, 12], F32, tag="st6")
                nc.vector.bn_stats(st6[:, 0:6], st.x[:, 0:512])
                nc.vector.bn_stats(st6[:, 6:12], st.x[:, 512:1024])
                mv = statp.tile([128, 2], F32, tag="mv")
                nc.vector.bn_aggr(mv[:], st6[:])
                lnv = statp.tile([128, 1], F32, tag="lnv")
                rstd = statp.tile([128, 1], F32, tag="rstd")
                # rstd = exp(-0.5*ln(var+eps)); Ln+Exp share one ACT table set
                nc.scalar.activation(lnv[:], mv[:, 1:2], AF.Ln, bias=eps_t[:])
                nc.scalar.activation(rstd[:], lnv[:], AF.Exp, scale=-0.5)
                xc = lnp.tile([128, D], F32, tag="xc", bufs=1)
                nc.vector.scalar_tensor_tensor(
                    xc[:], st.x[:], mv[:, 0:1], g_re[:],
                    op0=ALU.subtract, op1=ALU.mult)
                # write into the dead hqt tile, then DMA out
                nc.vector.scalar_tensor_tensor(
                    st.hqt[:], xc[:], rstd[:], b_re[:],
                    op0=ALU.mult, op1=ALU.add)
                nc.sync.dma_start(out_d[tt * 128:(tt + 1) * 128, :], st.hqt[:])

            def o_chain(tt, dci):
                d0 = dci * 512
                ch = SimpleNamespace(ps=None)

                def mm(fc):
                    def f():
                        if ch.ps is None:
                            if tt not in tt_state:
                                hqt = hqp.tile([128, D], F32, tag="hq")
                                nc.sync.dma_start(
                                    hqt[:], hq_d[tt * 128:(tt + 1) * 128, :])
                                x = lnp.tile([128, D], F32, tag="x")
                                tt_state[tt] = SimpleNamespace(
                                    hqt=hqt, x=x, done=0)
                            ch.ps = psP.tile([128, 512], F32, tag="P",
                                             name="ops")
                        nc.tensor.matmul(
                            ch.ps[:],
                            AVT[:, fc:fc + 2, tt * 128:(tt + 1) * 128],
                            woT[:, fc:fc + 2, d0: d0 + 512],
                            start=(fc == 0), stop=(fc == FC - 2),
                            perf_mode=DR, skip_group_check=True)
                    return f

                def fin():
                    st = tt_state[tt]
                    # residual add; 1/WSC^2 undoes the V and O weight scales
                    nc.vector.scalar_tensor_tensor(
                        st.x[:, d0:d0 + 512], ch.ps[:], 1.0 / (WSC * WSC),
                        st.hqt[:, d0:d0 + 512], op0=ALU.mult, op1=ALU.add)
                    st.done += 1
                    if st.done == 2:
                        ln_tail(tt, st)
                return SimpleNamespace(mms=deque(mm(fc) for fc in range(0, FC, 2)),
                                       fin=fin)

            # ---- build the drain queue in deadline order ----
            # Every chain carries a `deadline` (unit_idx, loop_iter): its data
            # is consumed by the attention emission at that position, so it
            # must be FULLY emitted before then (Tile tracks deps in program
            # order — a read emitted before its producing write races).
            # Chains whose matmuls depend on the attention pipeline itself
            # (the O-proj chains reading AVT) carry a `gate`: pump() must not
            # emit them until that many units are fully emitted, else the
            # in-order PE queue deadlocks on its own later instructions.
            # unit order (ic, hp); index in this list is the deadline key.
            units = [(0, 0), (0, 1), (0, 2), (0, 3),
                     (1, 0), (1, 1), (1, 2), (1, 3),
                     (0, 4), (0, 5), (0, 6), (0, 7),
                     (1, 4), (1, 5), (1, 6), (1, 7)]
            uidx_of = {u: i for i, u in enumerate(units)}
            LAG = 2
            FAR = (99, 0)

            kchunks = _chunks(TK, 512)
            queue = deque()
            units_done = SimpleNamespace(n=0)
            qmm = SimpleNamespace(n=0)

            def push(ch, deadline=FAR, gate=0):
                ch.gate = gate
                ch.deadline = deadline
                qmm.n += len(ch.mms)
                queue.append(ch)

            def k_dl(hp, ci):
                return (uidx_of[(0, hp)], (kchunks[ci][0]) // 128)

            def q_dl(hp, icx):
                return (uidx_of[(icx, hp)], 0)

            def v_dl(jc, fh):
                return (uidx_of[(0, 0 if fh == 0 else 4)], jc + LAG)

            # u1 needs: V(4..KC-1, fh0), KT(0,c1)
            fh0_rest = list(range(4, KC))
            if fh0_rest:
                push(v_chain(fh0_rest[0], 0), v_dl(fh0_rest[0], 0))
            for ci in range(1, len(kchunks)):
                push(k_chain(0, *kchunks[ci]), k_dl(0, ci))
            for jc in fh0_rest[1:]:
                push(v_chain(jc, 0), v_dl(jc, 0))
            # u2..u4: KT/QT for hp 1..3 (+ first V fh1 chains)
            for hp in (1, 2, 3):
                push(k_chain(hp, *kchunks[0]), k_dl(hp, 0))
                push(q_chain(hp, 0), q_dl(hp, 0))
                for ci in range(1, len(kchunks)):
                    push(k_chain(hp, *kchunks[ci]), k_dl(hp, ci))
                if hp >= 2:
                    push(v_chain(hp - 2, 1), v_dl(hp - 2, 1))
            # u5..u8: QT(0..3, i1), rest of V fh1, KT/QT hp4
            push(q_chain(0, 1), q_dl(0, 1))
            push(v_chain(2, 1), v_dl(2, 1))
            push(q_chain(1, 1), q_dl(1, 1))
            push(v_chain(3, 1), v_dl(3, 1))
            push(q_chain(2, 1), q_dl(2, 1))
            push(v_chain(4 % KC, 1), v_dl(4 % KC, 1))
            push(k_chain(4, *kchunks[0]), k_dl(4, 0))
            push(q_chain(4, 0), q_dl(4, 0))
            push(q_chain(3, 1), q_dl(3, 1))
            for ci in range(1, len(kchunks)):
                push(k_chain(4, *kchunks[ci]), k_dl(4, ci))
            for jc in range(5, KC):
                push(v_chain(jc, 1), v_dl(jc, 1))
            # u9..u12: KT/QT hp 5..7
            for hp in (5, 6, 7):
                push(k_chain(hp, *kchunks[0]), k_dl(hp, 0))
                push(q_chain(hp, 0), q_dl(hp, 0))
                for ci in range(1, len(kchunks)):
                    push(k_chain(hp, *kchunks[ci]), k_dl(hp, ci))
            push(q_chain(4, 1), q_dl(4, 1))
            # u13..u16: QT(5..7, i1) + O(ic0) (gated on AVT(ic0) complete)
            for hp in (5, 6, 7):
                push(q_chain(hp, 1), q_dl(hp, 1))
            for tt in range(0, ICS // 128):
                push(o_chain(tt, 0), FAR, gate=12)
                push(o_chain(tt, 1), FAR, gate=12)
            # tail: O(ic1) (gated on AVT(ic1) complete)
            for tt in range(ICS // 128, TQ // 128):
                push(o_chain(tt, 0), FAR, gate=16)
                push(o_chain(tt, 1), FAR, gate=16)

            def run_rest(ch):
                while ch.mms:
                    ch.mms.popleft()()
                    qmm.n -= 1
                ch.fin()

            def drain_due(pos):
                """Fully emit every queued chain whose deadline has arrived."""
                due = [ch for ch in queue if ch.deadline <= pos]
                for ch in due:
                    queue.remove(ch)
                    run_rest(ch)

            def pump(n):
                while n > 0 and queue:
                    ch = queue[0]
                    if ch.gate > units_done.n:
                        return
                    ch.mms.popleft()()
                    qmm.n -= 1
                    n -= 1
                    if not ch.mms:
                        ch.fin()
                        queue.popleft()

            # ---- prologue projections (emitted whole, never queued) ----
            def run_chain(ch):
                while ch.mms:
                    ch.mms.popleft()()
                ch.fin()

            run_chain(q_chain(0, 0))
            run_chain(k_chain(0, *kchunks[0]))
            for jc in range(min(4, KC)):
                run_chain(v_chain(jc, 0))

            # late loads (overlap with attention)
            for fc_ in range(FC):
                nc.sync.dma_start(sl(woT, fc_, 0, D),
                                  woT_d[fc_ * 128:(fc_ + 1) * 128, :])
            nc.sync.dma_start(g_re[:], g_d[:])
            nc.sync.dma_start(b_re[:], b_d[:])

            # ================= attention units =================
            slots = SimpleNamespace(n=len(units) * KC)

            for uidx, (icx, hp) in enumerate(units):
                io = icx * ICS
                pvA = psPV.tile([128, ICS], F32, tag="pv", name="pvA")
                pvB = psPV.tile([128, ICS], F32, tag="pv", name="pvB")
                e_pairs = {}

                def emit_S(jc):
                    s = psS.tile([128, 2 * ICS], F32, tag="S", name="s")
                    nc.tensor.matmul(
                        s[:, 0:ICS],
                        KT[0:64, hp * TK + jc * 128: hp * TK + (jc + 1) * 128],
                        QT[0:64, hp * TQ + io: hp * TQ + io + ICS],
                        start=True, stop=True, tile_position=(0, 0),
                        skip_group_check=True)
                    nc.tensor.matmul(
                        s[:, ICS:2 * ICS],
                        KT[64:128, hp * TK + jc * 128: hp * TK + (jc + 1) * 128],
                        QT[64:128, hp * TQ + io: hp * TQ + io + ICS],
                        start=True, stop=True, tile_position=(64, 0),
                        skip_group_check=True)
                    if jc % 2 == 0:
                        e_pairs[jc // 2] = ep.tile([128, 2, 2 * ICS], FP8E5,
                                                   tag="e", name="e")
                    e = e_pairs[jc // 2]
                    # scale folds attention 1/sqrt(dh) and the x16 fp8
                    # pre-scales of wq and wk back out of the raw scores.
                    # fp8e5 exp output feeds the DoubleRow PV matmul.
                    nc.scalar.activation(e[:, jc % 2:jc % 2 + 1, :].squeeze(1),
                                         s[:], AF.Exp,
                                         bias=mb[:, jc:jc + 1],
                                         scale=1.0 / (WSC * WSC * DH ** 0.5))

                def emit_PV(p2):
                    # DoubleRow over the key pair; ones inside V8 make the
                    # spare output partitions accumulate the denominators.
                    e = e_pairs.pop(p2)
                    st, sp = (p2 == 0), (p2 == KC2 - 1)
                    nc.tensor.matmul(
                        pvA[:, :],
                        V8[:, p2:p2 + 1, :, hp:hp + 1, 0:128]
                        .squeeze(1).squeeze(2),
                        e[:, :, 0:ICS], start=st, stop=sp,
                        perf_mode=DR, skip_group_check=True)
                    nc.tensor.matmul(
                        pvB[:, :],
                        V8[:, p2:p2 + 1, :, hp:hp + 1, 128:256]
                        .squeeze(1).squeeze(2),
                        e[:, :, ICS:2 * ICS], start=st, stop=sp,
                        perf_mode=DR, skip_group_check=True)

                for jc in range(KC + LAG):
                    # force-emit any chain whose consumer is imminent
                    drain_due((uidx, jc))
                    if jc < KC:
                        emit_S(jc)
                        # pace the projection queue across the whole kernel
                        rate = -(-qmm.n // max(slots.n, 1))
                        pump(min(rate, 6))
                        slots.n -= 1
                    else:
                        pump(2)
                    if jc >= LAG + 1 and (jc - LAG) % 2 == 1:
                        emit_PV((jc - LAG) // 2)

                # normalize -> AVT. pvA = [PV_h0 ; den_h0], pvB = [den_h1 ;
                # PV_h1]. Partition-shifted copies gather both denominators
                # into one tile, one approx-reciprocal, two multiplies.
                den = pvs.tile([128, ICS], F32, tag="pd")
                nc.vector.tensor_copy(den[0:64, :], pvA[64:128, :])
                nc.vector.tensor_copy(den[64:128, :], pvB[0:64, :])
                rec = pvs.tile([128, ICS], F32, tag="rc")
                nc.vector.reciprocal_approx_fast(rec[:], den[:])
                av3 = AVT[:, hp:hp + 1, io: io + ICS].squeeze(1)
                nc.vector.tensor_tensor(
                    av3[0:64, :], pvA[0:64, :], rec[0:64, :], op=ALU.mult)
                nc.vector.tensor_tensor(
                    av3[64:128, :], pvB[64:128, :], rec[64:128, :],
                    op=ALU.mult)
                units_done.n += 1

            # flush the deferred LayerNorms first: they run on DVE while the
            # PE starts the tail O-proj matmuls below
            ln_defer.on = False
            for tt_, st_ in ln_defer.q:
                ln_tail(tt_, st_)
            ln_defer.q.clear()
            # drain whatever projection work remains (O(ic1) mostly)
            pump(10 ** 9)

    nc.compile()
    return nc


def choose_tk(attn_mask):
    """Compacted key count: max unmasked count over batches, ceil to 256
    (the PV DoubleRow path pairs 128-key tiles)."""
    m = np.asarray(attn_mask)
    counts = (~m).sum(axis=0)
    tk = int(((int(counts.max()) + 255) // 256) * 256)
    return max(tk, 256)


def host_prep_core(c, tk, h, attn_mask, wq, wkv, wo, ln_g, ln_b, NH=16, DH=64):
    """Build the per-core input map (numpy) for core c."""
    T, B, D = h.shape
    F = NH * DH
    TQ = T // 2
    KC = tk // 128
    b, qh = c // 2, c % 2
    f8 = ml_dtypes.float8_e4m3
    hb = np.roll(np.asarray(h[:, b, :], dtype=np.float32), -qh * TQ, axis=0)
    maskb = np.roll(np.asarray(attn_mask[:, b]), -qh * TQ)
    idx = np.nonzero(~maskb)[0]
    nk = idx.shape[0]
    assert nk <= tk
    idxp = np.concatenate([idx, np.zeros(tk - nk, np.int64)])
    hbT = np.ascontiguousarray(hb.T).astype(f8)             # [D, T]
    m = {}
    m["hTq"] = np.ascontiguousarray(hbT[:, :TQ])
    m["hTk"] = np.ascontiguousarray(hbT[:, idxp])
    m["hq"] = np.ascontiguousarray(hb[:TQ])                 # [TQ, D] f32
    # weights stored x16 in fp8 (see WSC); compensated in-kernel
    m["wqT"] = np.ascontiguousarray(wq.T * WSC).astype(f8)
    m["wkT"] = np.ascontiguousarray(wkv[:F].T * WSC).astype(f8)
    m["wvT"] = np.ascontiguousarray(wkv[F:].T * WSC).astype(f8)
    m["woT"] = np.ascontiguousarray(wo.T * WSC).astype(f8)
    mbias = np.full(tk, NEG_BIG, np.float32)
    mbias[:nk] = 0.0
    m["maskbias"] = np.ascontiguousarray(mbias.reshape(KC, 128).T)
    m["g_rep"] = np.ascontiguousarray(
        np.broadcast_to(np.asarray(ln_g, np.float32), (128, D)))
    m["b_rep"] = np.ascontiguousarray(
        np.broadcast_to(np.asarray(ln_b, np.float32), (128, D)))
    return m

# ======================================================================
# Host-side runner: shard, compile (cached), execute on 8 cores, gather.
# ======================================================================
_NC_CACHE = {}
LAST_RESULT = None  # BassKernelResults of the most recent kernel() call


def _get_nc(T, TQ, TK, D, NH, DH):
    key = (T, TQ, TK, D, NH, DH)
    if key not in _NC_CACHE:
        _NC_CACHE[key] = build_nc(T, TQ, TK, D, NH, DH, n_cores=8, debug=False)
    return _NC_CACHE[key]


def kernel(h, attn_mask, wq, wkv, wo, ln_g, ln_b):
    """Full-input MultiHeadAttn forward on 8 NeuronCores.

    h: [T, B, D] f32; attn_mask: [T, B] bool (True = masked key);
    wq: [F, D]; wkv: [2F, D]; wo: [D, F]; ln_g/ln_b: [D].
    Returns [T, B, D] f32 = layer_norm(h + attn(h)).
    """
    from concourse.bass_utils import run_bass_kernel_spmd
    global LAST_RESULT

    h = np.asarray(h)
    attn_mask = np.asarray(attn_mask)
    wq = np.asarray(wq, np.float32)
    wkv = np.asarray(wkv, np.float32)
    wo = np.asarray(wo, np.float32)
    ln_g = np.asarray(ln_g, np.float32)
    ln_b = np.asarray(ln_b, np.float32)

    T, B, D = h.shape
    NH = 16
    DH = wq.shape[0] // NH
    assert 2 * B == 8, "sharding assumes batch 4 over 8 cores"
    TQ = T // 2
    TK = min(choose_tk(attn_mask), T)

    nc = _get_nc(T, TQ, TK, D, NH, DH)
    in_maps = [host_prep_core(c, TK, h, attn_mask, wq, wkv, wo, ln_g, ln_b,
                              NH=NH, DH=DH) for c in range(8)]
    res = run_bass_kernel_spmd(nc, in_maps, core_ids=list(range(8)))
    LAST_RESULT = res

    out = np.empty((T, B, D), np.float32)
    for c in range(8):
        b, qh = c // 2, c % 2
        out[qh * TQ:(qh + 1) * TQ, b, :] = res.results[c]["out"]
    return out
